# revision 16
# baseline (speedup 1.0000x reference)
"""CrossViewSwapAttention Trainium2 kernel.

Strategy (8 NeuronCores, SPMD, no collectives):
  Launch 1 (prep, sharded by feature rows / BEV rows):
    - geometry embeddings (img_embed, bev_embed) via folded 4xC matmuls
    - BN+ReLU+1x1conv for key/value features (BN folded into conv weights)
    - width-axis LayerNorm (stats per (ch,row) via bn_stats; its affine
      vanishes inside the following feature LayerNorm)
    - feature-dim LayerNorm "core" (x-mean)/std via ones-matmul broadcast
      trick (mean/E[x^2] broadcast over partitions by a 128x128 ones lhsT)
    -> outputs normalized K / V / Q token tensors (bf16, channel-major)
  Host: reshard tokens into per-core attention windows (4 windows/core).
  Launch 2 (windowed attention, sharded by (x,y) windows):
    - per window: project q/k/v (LN gains + tau folded into weights),
      scores^T = kh^T qh via head-row-tiled matmuls (4 heads share the PE),
      exp on ScalarE straight out of PSUM (one op per [128, 4*512] block),
      Z via ones-matmul column sums (col-tiled by head), PV col-tiled by
      head, normalize with reciprocal_approx_fast, out-proj with the
      mean-over-views folded into PSUM accumulation, MLPs, final LN.

All matmuls bf16 (fp32 runs at 1/4 rate on the PE); stats/normalization
paths fp32. All parameter-side algebra (LN gains, BN, tau, biases through
projections) is folded on the host; per-pixel/token compute is on-device.
"""

import numpy as np
from contextlib import ExitStack

import ml_dtypes

import concourse.bass as bass
import concourse.bacc as bacc
import concourse.tile as tile
from concourse import mybir
from concourse.bass_utils import run_bass_kernel_spmd

F32 = mybir.dt.float32
BF16 = mybir.dt.bfloat16
AF = mybir.ActivationFunctionType
ALU = mybir.AluOpType
BF = ml_dtypes.bfloat16

# problem constants
NV = 6
C = 128
FH, FW = 32, 128
IMG_H, IMG_W = 512, 1408
H, W = 64, 128
HEADS, DH = 4, 32
NCORE = 8
EPS = 1e-5

# per-core shard sizes
RPC = FH // NCORE            # 4 feature rows per view per core
HPC = H // NCORE             # 8 bev rows per core
WPC = 4                      # windows per core
NQ1 = NV * 256               # 1536 q tokens per layer-1 window
NK = NV * 128                # 768 k tokens per window
NQ2 = 256                    # deduped q tokens per layer-2 window


# ---------------------------------------------------------------------------
# device-side helpers
# ---------------------------------------------------------------------------

def _featln(nc, pools, x_sb, n_tok, out_sb, ones_neg, ones_pos, chunk=512,
            mult_gp=False):
    """out = (x - mean(x)) / sqrt(var(x) + EPS), mean/var over the 128
    partitions (feature dim), per token (free dim). x_sb/out_sb: [128, n_tok].
    ones_neg/ones_pos: [128,128] bf16 lhsT tiles of -1/128, +1/128. Stats are
    computed on a bf16 copy (1-cycle-per-row matmuls); the normalization
    itself reads the original x. mult_gp routes the final multiply to the
    (otherwise idle) GpSimd engine."""
    sb, lnp = pools["sb_ln"], pools["ps_ln"]
    CH = chunk
    for i in range(0, n_tok, CH):
        n = min(CH, n_tok - i)
        xc = x_sb[:, i:i + n]
        if x_sb.dtype == BF16:
            xb = xc
        else:
            xbt = sb.tile([128, CH], BF16, tag="ln_xb")
            nc.vector.tensor_copy(out=xbt[:, :n], in_=xc)
            xb = xbt[:, :n]
        sq = sb.tile([128, CH], BF16, tag="ln_sq")
        nc.scalar.square(sq[:, :n], xb)
        negm = lnp.tile([128, CH], F32, tag="ln_negm")
        esq = lnp.tile([128, CH], F32, tag="ln_esq")
        for j in range(0, n, 512):
            m = min(512, n - j)
            nc.tensor.matmul(negm[:, j:j + m], ones_neg, xb[:, j:j + m],
                             start=True, stop=True)
            nc.tensor.matmul(esq[:, j:j + m], ones_pos, sq[:, j:j + m],
                             start=True, stop=True)
        msq = sb.tile([128, CH], F32, tag="ln_msq")
        nc.scalar.square(msq[:, :n], negm[:, :n])
        # nve = m^2 - eps - E[x^2] = -(var + eps)
        nve = sb.tile([128, CH], F32, tag="ln_nve")
        nc.vector.scalar_tensor_tensor(
            out=nve[:, :n], in0=msq[:, :n], scalar=EPS, in1=esq[:, :n],
            op0=ALU.subtract, op1=ALU.subtract)
        rv = sb.tile([128, CH], F32, tag="ln_rv")
        nc.vector.reciprocal_approx_fast(out=rv[:, :n], in_=nve[:, :n])
        rsig = sb.tile([128, CH], F32, tag="ln_rsig")
        nc.scalar.activation(out=rsig[:, :n], in_=rv[:, :n], func=AF.Sqrt,
                             scale=-1.0)
        cent = sb.tile([128, CH], F32, tag="ln_cent")
        nc.vector.tensor_tensor(out=cent[:, :n], in0=xc, in1=negm[:, :n],
                                op=ALU.add)
        eng = nc.gpsimd if mult_gp else nc.vector
        eng.tensor_tensor(out=out_sb[:, i:i + n], in0=cent[:, :n],
                          in1=rsig[:, :n], op=ALU.mult)


def _mlp(nc, pools, x_sb, n_tok, wa, ba, wb, bb, ones_neg_f, ones_pos_f,
         out_sb):
    """out = x + W2^T gelu(W1^T LN(x) + b1) + b2 (residual included).
    x_sb f32 [128, n_tok]; wa [128,2,128] bf16; ba [128,2]; wb [128,2,128];
    bb [128,1]."""
    sb, sm = pools["sb_ln"], pools["ps_sm"]
    xn = sb.tile([128, n_tok], BF16, tag="mlp_xn")
    _featln(nc, pools, x_sb, n_tok, xn, ones_neg_f, ones_pos_f)
    h_sb = sb.tile([128, 2, n_tok], BF16, tag="mlp_h")
    for j in range(2):
        for sp in range(0, n_tok, 512):
            m = min(512, n_tok - sp)
            hp = sm.tile([128, 512], F32, tag="sm")
            nc.tensor.matmul(hp[:, :m], wa[:, j, :], xn[:, sp:sp + m],
                             start=True, stop=True)
            nc.scalar.activation(out=h_sb[:, j, sp:sp + m], in_=hp[:, :m],
                                 func=AF.Gelu, bias=ba[:, j:j + 1], scale=1.0)
    for sp in range(0, n_tok, 512):
        m = min(512, n_tok - sp)
        yp = sm.tile([128, 512], F32, tag="sm")
        for j in range(2):
            nc.tensor.matmul(yp[:, :m], wb[:, j, :], h_sb[:, j, sp:sp + m],
                             start=(j == 0), stop=(j == 1))
        nc.vector.scalar_tensor_tensor(
            out=out_sb[:, sp:sp + m], in0=yp[:, :m], scalar=bb,
            in1=x_sb[:, sp:sp + m], op0=ALU.add, op1=ALU.add)


# ---------------------------------------------------------------------------
# launch 1: prep
# ---------------------------------------------------------------------------

def build_prep():
    nc = bacc.Bacc("TRN2", target_bir_lowering=False, debug=False,
                   num_devices=NCORE)
    d = {}
    def di(name, shape, dt):
        d[name] = nc.dram_tensor(name, shape, dt, kind="ExternalInput").ap()
    def do(name, shape, dt):
        d[name] = nc.dram_tensor(name, shape, dt, kind="ExternalOutput").ap()

    di("feat", [128, NV * RPC, FW], F32)      # feature rows (v-major)
    di("geomw", [4, NV, 128], F32)            # per-view [A_v; b_v - c_v] lhsT
    di("pixaug", [4, RPC, FW], F32)           # [x*1408, y*512, 1, 1] rows
    di("wbevt", [3, NV, 128], BF16)           # per view [W_bev^T; b_bev-c_v]
    di("gridaug", [3, HPC * W], BF16)         # [grid0 rows; ones]
    di("xrows", [128, HPC, W], F32)           # x, this core's bev rows
    di("betafp", [128, 1], F32)
    di("betafl", [128, 1], F32)
    di("wfp", [128, 128], BF16)               # (W_fp * a_bn).T lhsT
    di("wfl", [128, 128], BF16)
    di("gatet", [128, 1], F32)                # embed_gate per partition

    do("xnq", [128, NV, HPC, W], BF16)
    do("xnk", [128, NV * RPC, FW], BF16)
    do("xnv", [128, NV * RPC, FW], BF16)

    with tile.TileContext(nc) as tc:
        with ExitStack() as ctx:
            _prep_body(ctx, tc, d)
    nc.compile()
    return nc


def _prep_body(ctx, tc, d):
    nc = tc.nc
    const = ctx.enter_context(tc.tile_pool(name="const", bufs=1))
    sb = ctx.enter_context(tc.tile_pool(name="sb", bufs=1))
    sb2 = ctx.enter_context(tc.tile_pool(name="sb2", bufs=2))
    pp = ctx.enter_context(tc.tile_pool(name="pp", bufs=2, space="PSUM"))
    lnp = ctx.enter_context(tc.tile_pool(name="lnp", bufs=1, space="PSUM"))
    pools = {"sb_ln": sb2, "ps_ln": lnp, "ps_sm": pp}

    # constants
    ones1_b = const.tile([128, 128], BF16)
    nc.vector.memset(ones1_b, 1.0)
    oneg_b = const.tile([128, 128], BF16)
    nc.vector.memset(oneg_b, -1.0 / 128)
    opos_b = const.tile([128, 128], BF16)
    nc.vector.memset(opos_b, 1.0 / 128)

    cnst = {}
    for nm in ("geomw", "pixaug", "wbevt", "gridaug", "xrows",
               "betafp", "betafl", "wfp", "wfl", "gatet", "feat"):
        ap = d[nm]
        cnst[nm] = const.tile(list(ap.shape), ap.dtype, name=nm + "_sb")
        nc.sync.dma_start(out=cnst[nm], in_=ap)

    # ---- BEV / query side -------------------------------------------------
    # per view: bev_v = W_bev@grid + (b_bev - c_v), normalize, + x
    NPOS = HPC * W  # 1024 positions
    q = sb.tile([128, NV, NPOS], BF16)
    xflat = cnst["xrows"].rearrange("p a b -> p (a b)")
    for v in range(NV):
        wps = pp.tile([128, NPOS], F32, tag="pp")
        for j in range(0, NPOS, 512):
            nc.tensor.matmul(wps[:, j:j + 512], cnst["wbevt"][:, v, :],
                             cnst["gridaug"][:, j:j + 512],
                             start=True, stop=True)
        sqv = sb2.tile([128, NPOS], BF16, tag="bev_sq")
        nc.scalar.square(sqv, wps)
        n2 = pp.tile([128, NPOS], F32, tag="pp")
        for j in range(0, NPOS, 512):
            nc.tensor.matmul(n2[:, j:j + 512], ones1_b, sqv[:, j:j + 512],
                             start=True, stop=True)
        rn2 = sb2.tile([128, NPOS], F32, tag="bev_rn2")
        nc.vector.reciprocal_approx_fast(out=rn2, in_=n2)
        rn = sb2.tile([128, NPOS], F32, tag="bev_rn")
        nc.scalar.activation(out=rn, in_=rn2, func=AF.Sqrt, scale=1.0)
        qp = sb2.tile([128, NPOS], BF16, tag="bev_qp")
        nc.vector.tensor_tensor(out=qp, in0=wps, in1=rn, op=ALU.mult)
        nc.gpsimd.tensor_tensor(out=q[:, v, :], in0=qp, in1=xflat,
                                op=ALU.add)
    xnq = sb.tile([128, NV * NPOS], BF16)
    _featln(nc, pools, q.rearrange("p a b -> p (a b)"), NV * NPOS, xnq,
            oneg_b, opos_b, chunk=1024, mult_gp=True)
    nc.sync.dma_start(out=d["xnq"],
                      in_=xnq.rearrange("p (a b c) -> p a b c", b=HPC, c=W))

    # ---- image / key / value side ----------------------------------------
    # d_embed per view rows, normalized -> img (bf16)
    img = sb.tile([128, NV, RPC, FW], BF16)
    for v in range(NV):
        dps = pp.tile([128, RPC, FW], F32, tag="pp")
        for r in range(RPC):
            nc.tensor.matmul(dps[:, r, :], cnst["geomw"][:, v, :],
                             cnst["pixaug"][:, r, :], start=True, stop=True)
        dpsf = dps.rearrange("p a b -> p (a b)")
        sqi = sb2.tile([128, RPC * FW], BF16, tag="img_sq")
        nc.scalar.square(sqi, dpsf)
        n2i = pp.tile([128, RPC * FW], F32, tag="pp")
        nc.tensor.matmul(n2i, ones1_b, sqi, start=True, stop=True)
        rn2i = sb2.tile([128, RPC * FW], F32, tag="img_rn2")
        nc.vector.reciprocal_approx_fast(out=rn2i, in_=n2i)
        rni = sb2.tile([128, RPC * FW], F32, tag="img_rn")
        nc.scalar.activation(out=rni, in_=rn2i, func=AF.Sqrt, scale=1.0)
        nc.vector.tensor_tensor(
            out=img[:, v, :, :].rearrange("p a b -> p (a b)"),
            in0=dpsf, in1=rni, op=ALU.mult)

    # relu(feat + beta) -> bf16
    NPX = NV * RPC * FW  # 3072
    featf = cnst["feat"].rearrange("p a b -> p (a b)")
    tfp = sb.tile([128, NPX], BF16)
    nc.vector.tensor_scalar(out=tfp, in0=featf, scalar1=cnst["betafp"],
                            scalar2=0.0, op0=ALU.add, op1=ALU.max)
    tfl = sb.tile([128, NPX], BF16)
    nc.vector.tensor_scalar(out=tfl, in0=featf, scalar1=cnst["betafl"],
                            scalar2=0.0, op0=ALU.add, op1=ALU.max)

    # convs + img gate -> key_flat / val_flat (bf16)
    imgf = img.rearrange("p a b c -> p (a b c)")
    kf = sb.tile([128, NV * RPC, FW], BF16)
    vf = sb.tile([128, NV * RPC, FW], BF16)
    kff = kf.rearrange("p a b -> p (a b)")
    vff = vf.rearrange("p a b -> p (a b)")
    for j in range(0, NPX, 512):
        kc = pp.tile([128, 512], F32, tag="pp")
        nc.tensor.matmul(kc, cnst["wfp"], tfp[:, j:j + 512], start=True,
                         stop=True)
        nc.vector.scalar_tensor_tensor(
            out=kff[:, j:j + 512], in0=imgf[:, j:j + 512],
            scalar=cnst["gatet"], in1=kc, op0=ALU.mult, op1=ALU.add)
        vc = pp.tile([128, 512], F32, tag="pp")
        nc.tensor.matmul(vc, cnst["wfl"], tfl[:, j:j + 512], start=True,
                         stop=True)
        nc.scalar.copy(out=vff[:, j:j + 512], in_=vc)

    # width-axis LN (affine part vanishes into the following feature LN)
    NR = NV * RPC  # 24 rows
    def width_ln(src, dst):
        mv = sb2.tile([128, NR, 2], F32, tag="wln_mv")
        for r in range(NR):
            stats = sb2.tile([128, 6], F32, tag="wln_stats")
            nc.vector.bn_stats(out=stats, in_=src[:, r, :])
            nc.vector.bn_aggr(out=mv[:, r, :], in_=stats)
        ve = sb2.tile([128, NR], F32, tag="wln_ve")
        nc.vector.tensor_scalar(out=ve, in0=mv[:, :, 1], scalar1=EPS,
                                scalar2=None, op0=ALU.add)
        rv = sb2.tile([128, NR], F32, tag="wln_rv")
        nc.vector.reciprocal_approx_fast(out=rv, in_=ve)
        rs = sb2.tile([128, NR], F32, tag="wln_rs")
        nc.scalar.activation(out=rs, in_=rv, func=AF.Sqrt, scale=1.0)
        for r in range(NR):
            nc.vector.tensor_scalar(
                out=dst[:, r, :], in0=src[:, r, :],
                scalar1=mv[:, r, 0:1], scalar2=rs[:, r:r + 1],
                op0=ALU.subtract, op1=ALU.mult)

    kbar = sb.tile([128, NR, FW], BF16)
    width_ln(kf, kbar)
    vbar = sb.tile([128, NR, FW], BF16)
    width_ln(vf, vbar)

    # feature-dim LN -> outputs
    xnk = sb.tile([128, NV * RPC * FW], BF16)
    _featln(nc, pools, kbar.rearrange("p a b -> p (a b)"), NPX, xnk,
            oneg_b, opos_b, chunk=1024, mult_gp=True)
    nc.sync.dma_start(out=d["xnk"],
                      in_=xnk.rearrange("p (a b) -> p a b", b=FW))
    xnv = sb.tile([128, NV * RPC * FW], BF16)
    _featln(nc, pools, vbar.rearrange("p a b -> p (a b)"), NPX, xnv,
            oneg_b, opos_b, chunk=1024, mult_gp=True)
    nc.sync.dma_start(out=d["xnv"],
                      in_=xnv.rearrange("p (a b) -> p a b", b=FW))


# ---------------------------------------------------------------------------
# launch 2: windowed attention
# ---------------------------------------------------------------------------

def build_attn():
    nc = bacc.Bacc("TRN2", target_bir_lowering=False, debug=False,
                   num_devices=NCORE)
    d = {}
    def di(name, shape, dt):
        d[name] = nc.dram_tensor(name, shape, dt, kind="ExternalInput").ap()
    def do(name, shape, dt):
        d[name] = nc.dram_tensor(name, shape, dt, kind="ExternalOutput").ap()

    di("xnq1w", [128, WPC, NQ1], BF16)
    di("xnk1w", [128, WPC, NK], BF16)
    di("xnv1w", [128, WPC, NK], BF16)
    di("xnk2w", [128, WPC, NK], BF16)
    di("xnv2w", [128, WPC, NK], BF16)
    di("skipw", [128, WPC, 256], F32)
    for nm in ("wq1", "wk1", "wv1", "wp1", "wq2", "wk2", "wv2", "wp2"):
        di(nm, [128, 128], BF16)
    for nm in ("bq1", "bk1", "bp1", "bq2", "bk2", "bp2", "bm1b", "bm2b",
               "postg", "postb"):
        di(nm, [128, 1], F32)
    di("wm1a", [128, 2, 128], BF16)
    di("wm1b", [128, 2, 128], BF16)
    di("wm2a", [128, 2, 128], BF16)
    di("wm2b", [128, 2, 128], BF16)
    di("bm1a", [128, 2], F32)
    di("bm2a", [128, 2], F32)

    do("outw", [128, WPC, 256], F32)

    with tile.TileContext(nc) as tc:
        with ExitStack() as ctx:
            _attn_body(ctx, tc, d)
    nc.compile()
    return nc


def _attn_body(ctx, tc, d):
    nc = tc.nc
    const = ctx.enter_context(tc.tile_pool(name="const", bufs=1))
    sb = ctx.enter_context(tc.tile_pool(name="sb", bufs=1))
    sb2 = ctx.enter_context(tc.tile_pool(name="sb2", bufs=2))
    win = ctx.enter_context(tc.tile_pool(name="win", bufs=2))
    ptp = ctx.enter_context(tc.tile_pool(name="ptp", bufs=2))
    qk = ctx.enter_context(tc.tile_pool(name="qk", bufs=1, space="PSUM"))
    sm = ctx.enter_context(tc.tile_pool(name="sm", bufs=2, space="PSUM"))
    lnp = ctx.enter_context(tc.tile_pool(name="lnp", bufs=1, space="PSUM"))
    pools = {"sb_ln": sb2, "ps_ln": lnp, "ps_sm": sm}

    cw = {}
    for nm, ap in d.items():
        if nm in ("outw",):
            continue
        cw[nm] = const.tile(list(ap.shape), ap.dtype, name=nm + "_sb")
        nc.sync.dma_start(out=cw[nm], in_=ap)
    ones1_b = const.tile([128, 128], BF16)
    nc.vector.memset(ones1_b, 1.0)
    oneg_f = const.tile([128, 128], BF16)
    nc.vector.memset(oneg_f, -1.0 / 128)
    opos_f = const.tile([128, 128], BF16)
    nc.vector.memset(opos_f, 1.0 / 128)

    q2all = sb.tile([128, WPC * 256], F32)

    # ---------------- layer 1 windows ----------------
    for w in range(WPC):
        an = _attention(nc, pools, sm, qk, ptp, win, ones1_b,
                        cw["xnq1w"][:, w, :], cw["xnk1w"][:, w, :],
                        cw["xnv1w"][:, w, :], NQ1,
                        cw["wq1"], cw["bq1"], cw["wk1"], cw["bk1"], cw["wv1"])
        # out-proj with mean over views folded into PSUM accumulation
        zm = sm.tile([128, 512], F32, tag="sm")
        for v in range(NV):
            nc.tensor.matmul(zm[:, :256], cw["wp1"],
                             an[:, v * 256:(v + 1) * 256],
                             start=(v == 0), stop=(v == NV - 1))
        nc.vector.scalar_tensor_tensor(
            out=q2all[:, w * 256:(w + 1) * 256], in0=zm[:, :256],
            scalar=cw["bp1"], in1=cw["skipw"][:, w, :],
            op0=ALU.add, op1=ALU.add)

    # ---------------- MLP 1 ----------------
    q2p = sb.tile([128, WPC * 256], F32)
    _mlp(nc, pools, q2all, WPC * 256, cw["wm1a"], cw["bm1a"], cw["wm1b"],
         cw["bm1b"], oneg_f, opos_f, q2p)

    # ---------------- layer 2 windows ----------------
    xnq2 = sb.tile([128, WPC * 256], BF16)
    _featln(nc, pools, q2p, WPC * 256, xnq2, oneg_f, opos_f)

    q3all = sb.tile([128, WPC * 256], F32)
    for w in range(WPC):
        an = _attention(nc, pools, sm, qk, ptp, win, ones1_b,
                        xnq2[:, w * 256:(w + 1) * 256],
                        cw["xnk2w"][:, w, :], cw["xnv2w"][:, w, :], NQ2,
                        cw["wq2"], cw["bq2"], cw["wk2"], cw["bk2"], cw["wv2"])
        zm = sm.tile([128, 512], F32, tag="sm")
        nc.tensor.matmul(zm[:, :256], cw["wp2"], an, start=True, stop=True)
        nc.vector.scalar_tensor_tensor(
            out=q3all[:, w * 256:(w + 1) * 256], in0=zm[:, :256],
            scalar=cw["bp2"], in1=q2p[:, w * 256:(w + 1) * 256],
            op0=ALU.add, op1=ALU.add)

    # ---------------- MLP 2 + post LN ----------------
    q3p = sb.tile([128, WPC * 256], F32)
    _mlp(nc, pools, q3all, WPC * 256, cw["wm2a"], cw["bm2a"], cw["wm2b"],
         cw["bm2b"], oneg_f, opos_f, q3p)

    xn3 = sb.tile([128, WPC * 256], F32)
    _featln(nc, pools, q3p, WPC * 256, xn3, oneg_f, opos_f)
    outw = sb.tile([128, WPC * 256], F32)
    nc.vector.tensor_scalar(out=outw, in0=xn3, scalar1=cw["postg"],
                            scalar2=cw["postb"], op0=ALU.mult, op1=ALU.add)
    nc.sync.dma_start(out=d["outw"],
                      in_=outw.rearrange("p (a b) -> p a b", b=256))


def _attention(nc, pools, sm, qk, ptp, win, ones1_b,
               xnq_sb, xnk_sb, xnv_sb, nq, wq, bq, wk, bk, wv):
    """One window of cross attention. Returns an [128, nq] bf16 tile with the
    normalized per-head attention output (channel-major, heads stacked)."""
    # qh = wq^T xnq + bq  (bf16, [128 hd, nq])
    qh = win.tile([128, nq], BF16, tag="qh")
    for sp in range(0, nq, 512):
        m = min(512, nq - sp)
        qp = sm.tile([128, 512], F32, tag="sm")
        nc.tensor.matmul(qp[:, :m], wq, xnq_sb[:, sp:sp + m],
                         start=True, stop=True)
        nc.vector.tensor_scalar(out=qh[:, sp:sp + m], in0=qp[:, :m],
                                scalar1=bq, scalar2=None, op0=ALU.add)
    # kh = wk^T xnk + bk
    kh = win.tile([128, NK], BF16, tag="kh")
    for sp in range(0, NK, 384):
        kp = sm.tile([128, 512], F32, tag="sm")
        nc.tensor.matmul(kp[:, :384], wk, xnk_sb[:, sp:sp + 384],
                         start=True, stop=True)
        nc.vector.tensor_scalar(out=kh[:, sp:sp + 384], in0=kp[:, :384],
                                scalar1=bk, scalar2=None, op0=ALU.add)
    # vh token-major: [128 tok, kc, hd]
    nkc = NK // 128  # 6 key chunks
    vh = win.tile([128, nkc, 128], BF16, tag="vh")
    for kc in range(nkc):
        vp = sm.tile([128, 512], F32, tag="sm")
        nc.tensor.matmul(vp[:, :128], xnv_sb[:, kc * 128:(kc + 1) * 128], wv,
                         start=True, stop=True)
        nc.vector.tensor_copy(out=vh[:, kc, :], in_=vp[:, :128])

    span = 512 if nq >= 512 else nq
    nsp = (nq + span - 1) // span
    an = win.tile([128, nq], BF16, tag="an")
    for s in range(nsp):
        q0 = s * span
        # scores^T -> exp(P^T) bf16, [128 k, kc, head, span]
        pt = ptp.tile([128, nkc, HEADS, span], BF16, tag="pt")
        for kc in range(nkc):
            for hp in range(2):
                sc = qk.tile([128, 2, 512], F32, tag="qk", bufs=2)
                for hh in range(2):
                    h = 2 * hp + hh
                    hs = slice(32 * h, 32 * h + 32)
                    nc.tensor.matmul(
                        sc[:, hh, :span],
                        kh[hs, kc * 128:(kc + 1) * 128],
                        qh[hs, q0:q0 + span],
                        start=True, stop=True, tile_position=(32 * h, 0))
                nc.scalar.activation(out=pt[:, kc, 2 * hp:2 * hp + 2, :],
                                     in_=sc[:, :, :span], func=AF.Exp)
        # Z (col-tiled ones-matmul) and PV (col-tiled by head)
        zp = sm.tile([128, 512], F32, tag="sm")
        pv = sm.tile([128, 512], F32, tag="sm")
        for kc in range(nkc):
            for h in range(HEADS):
                op = slice(32 * h, 32 * h + 32)
                nc.tensor.matmul(zp[op, :span], ones1_b[:, 0:32],
                                 pt[:, kc, h, :], start=(kc == 0),
                                 stop=(kc == nkc - 1),
                                 tile_position=(0, 32 * h))
                nc.tensor.matmul(pv[op, :span], vh[:, kc, op],
                                 pt[:, kc, h, :], start=(kc == 0),
                                 stop=(kc == nkc - 1),
                                 tile_position=(0, 32 * h))
        rz = win.tile([128, 512], F32, tag="rz")
        nc.vector.reciprocal_approx_fast(out=rz[:, :span], in_=zp[:, :span])
        nc.vector.tensor_tensor(out=an[:, q0:q0 + span], in0=pv[:, :span],
                                in1=rz[:, :span], op=ALU.mult)
    return an


# ---------------------------------------------------------------------------
# host orchestration
# ---------------------------------------------------------------------------

_PROGS = {}


def _progs():
    if "prep" not in _PROGS:
        _PROGS["prep"] = build_prep()
        _PROGS["attn"] = build_attn()
    return _PROGS["prep"], _PROGS["attn"]


def kernel(index, x, grid0, feature, I_inv, E_inv, object_count, params):
    p = {}
    for k, v in params.items():
        if k in ("a1", "a2", "mlp1", "mlp2"):
            p[k] = {kk: np.asarray(vv, dtype=np.float32)
                    for kk, vv in v.items()}
        else:
            p[k] = np.asarray(v, dtype=np.float32)
    x = np.asarray(x, dtype=np.float32)
    grid0 = np.asarray(grid0, dtype=np.float32)
    feature = np.asarray(feature, dtype=np.float32)
    I_inv = np.asarray(I_inv, dtype=np.float32)
    E_inv = np.asarray(E_inv, dtype=np.float32)

    oc = float(np.asarray(object_count).reshape(-1)[0])
    tau = float(np.clip(2.0 / (5.0 + max(oc, 0.0)) + 0.6, 0.4, 1.5))
    s = (DH ** -0.5) / tau

    prep, attn = _progs()

    # ---- geometry folds ----
    Wi, Wc = p["W_img"], p["W_cam"]
    geomw = np.zeros((4, NV, 128), np.float32)
    wbevt = np.zeros((3, NV, 128), np.float32)
    for v in range(NV):
        Ai = Wi @ E_inv[0, v, :, :3] @ I_inv[0, v]         # (128,3)
        bi = Wi @ E_inv[0, v, :, 3]                        # (128,)
        cv = Wc @ E_inv[0, v, :, 3]
        geomw[:3, v, :] = Ai.T
        geomw[3, v, :] = bi - cv
        wbevt[:2, v, :] = p["W_bev"].T
        wbevt[2, v, :] = p["b_bev"] - cv
    wbevt = wbevt.astype(BF)

    xs = np.linspace(0.0, 1.0, FW, dtype=np.float32) * IMG_W
    ys = np.linspace(0.0, 1.0, FH, dtype=np.float32) * IMG_H

    afp = p["bn_fp_g"] / np.sqrt(p["bn_fp_v"] + EPS)
    bfp = (p["bn_fp_b"] - p["bn_fp_m"] * afp) / afp
    afl = p["bn_fl_g"] / np.sqrt(p["bn_fl_v"] + EPS)
    bfl = (p["bn_fl_b"] - p["bn_fl_m"] * afl) / afl
    wfp = np.ascontiguousarray((p["W_fp"] * afp[None, :]).T).astype(BF)
    wfl = np.ascontiguousarray((p["W_fl"] * afl[None, :]).T).astype(BF)
    gate = float(p["embed_gate"])

    # ---- launch 1 ----
    in1 = []
    for c in range(NCORE):
        fh0 = RPC * c
        h0 = HPC * c
        pixaug = np.zeros((4, RPC, FW), np.float32)
        pixaug[0] = xs[None, :]
        pixaug[1] = ys[fh0:fh0 + RPC, None]
        pixaug[2] = 1.0
        pixaug[3] = 1.0
        gridaug = np.concatenate(
            [grid0[:, h0:h0 + HPC, :].reshape(2, -1),
             np.ones((1, HPC * W), np.float32)], 0).astype(BF)
        m = {
            "feat": np.ascontiguousarray(
                feature[0, :, :, fh0:fh0 + RPC, :].transpose(1, 0, 2, 3)
            ).reshape(128, NV * RPC, FW),
            "geomw": geomw, "pixaug": pixaug,
            "wbevt": wbevt, "gridaug": gridaug,
            "xrows": np.ascontiguousarray(x[0, :, h0:h0 + HPC, :]),
            "betafp": bfp.reshape(128, 1), "betafl": bfl.reshape(128, 1),
            "wfp": wfp, "wfl": wfl,
            "gatet": np.full((128, 1), gate, np.float32),
        }
        in1.append(m)
    res1 = run_bass_kernel_spmd(prep, in1, list(range(NCORE)))

    xnk = np.zeros((128, NV, FH, FW), np.float32)
    xnv = np.zeros((128, NV, FH, FW), np.float32)
    xnq = np.zeros((128, NV, H, W), np.float32)
    for c in range(NCORE):
        r = res1.results[c]
        xnk[:, :, RPC * c:RPC * (c + 1), :] = \
            r["xnk"].reshape(128, NV, RPC, FW).astype(np.float32)
        xnv[:, :, RPC * c:RPC * (c + 1), :] = \
            r["xnv"].reshape(128, NV, RPC, FW).astype(np.float32)
        xnq[:, :, HPC * c:HPC * (c + 1), :] = r["xnq"].astype(np.float32)

    # ---- attention weight folds ----
    def attw(a):
        wq = ((a["qln_g"][:, None] * a["Wq"]) * s).astype(BF)
        bqv = ((a["qln_b"] @ a["Wq"] + a["bq"]) * s).astype(np.float32)
        wk = (a["kln_g"][:, None] * a["Wk"]).astype(BF)
        bkv = (a["kln_b"] @ a["Wk"] + a["bk"]).astype(np.float32)
        wvm = (a["vln_g"][:, None] * a["Wv"]).astype(BF)
        bvv = (a["vln_b"] @ a["Wv"] + a["bv"]).astype(np.float32)
        bpv = (bvv @ a["Wp"] + a["bp"]).astype(np.float32)
        return wq, bqv, wk, bkv, wvm, bpv

    wq1, bq1, wk1, bk1, wv1, bp1 = attw(p["a1"])
    wq2, bq2, wk2, bk2, wv2, bp2 = attw(p["a2"])

    def mlpw(mp, g, b):
        wa = (g[:, None] * mp["W1"]).reshape(128, 2, 128).astype(BF)
        ba = np.ascontiguousarray(
            (b @ mp["W1"] + mp["b1"]).reshape(2, 128).T).astype(np.float32)
        wb = np.ascontiguousarray(
            mp["W2"].reshape(2, 128, 128).transpose(1, 0, 2)).astype(BF)
        bb = mp["b2"].reshape(128, 1).astype(np.float32)
        return wa, ba, wb, bb

    wm1a, bm1a, wm1b, bm1b = mlpw(p["mlp1"], p["pre1_g"], p["pre1_b"])
    wm2a, bm2a, wm2b, bm2b = mlpw(p["mlp2"], p["pre2_g"], p["pre2_b"])

    shared2 = {
        "wq1": wq1, "wk1": wk1, "wv1": wv1,
        "wp1": (p["a1"]["Wp"] / NV).astype(BF),
        "wq2": wq2, "wk2": wk2, "wv2": wv2,
        "wp2": p["a2"]["Wp"].astype(BF),
        "bq1": bq1.reshape(128, 1), "bk1": bk1.reshape(128, 1),
        "bp1": bp1.reshape(128, 1),
        "bq2": bq2.reshape(128, 1), "bk2": bk2.reshape(128, 1),
        "bp2": bp2.reshape(128, 1),
        "wm1a": wm1a, "wm1b": wm1b, "wm2a": wm2a, "wm2b": wm2b,
        "bm1a": bm1a, "bm2a": bm2a, "bm1b": bm1b, "bm2b": bm2b,
        "postg": p["post_g"].reshape(128, 1).astype(np.float32),
        "postb": p["post_b"].reshape(128, 1).astype(np.float32),
    }

    in2 = []
    wins = []
    for c in range(NCORE):
        xnq1w = np.zeros((128, WPC, NQ1), np.float32)
        xnk1w = np.zeros((128, WPC, NK), np.float32)
        xnv1w = np.zeros((128, WPC, NK), np.float32)
        xnk2w = np.zeros((128, WPC, NK), np.float32)
        xnv2w = np.zeros((128, WPC, NK), np.float32)
        skipw = np.zeros((128, WPC, 256), np.float32)
        cwins = []
        for j in range(WPC):
            wlin = WPC * c + j
            Xi, Yi = wlin // 8, wlin % 8
            cwins.append((Xi, Yi))
            xnq1w[:, j] = xnq[:, :, 16 * Xi:16 * Xi + 16,
                              16 * Yi:16 * Yi + 16].reshape(128, NQ1)
            xnk1w[:, j] = xnk[:, :, 8 * Xi:8 * Xi + 8,
                              16 * Yi:16 * Yi + 16].reshape(128, NK)
            xnv1w[:, j] = xnv[:, :, 8 * Xi:8 * Xi + 8,
                              16 * Yi:16 * Yi + 16].reshape(128, NK)
            xnk2w[:, j] = xnk[:, :, Xi::4, Yi::8].reshape(128, NK)
            xnv2w[:, j] = xnv[:, :, Xi::4, Yi::8].reshape(128, NK)
            skipw[:, j] = x[0, :, 16 * Xi:16 * Xi + 16,
                            16 * Yi:16 * Yi + 16].reshape(128, 256)
        wins.append(cwins)
        m = dict(shared2)
        m.update({
            "xnq1w": xnq1w.astype(BF), "xnk1w": xnk1w.astype(BF),
            "xnv1w": xnv1w.astype(BF), "xnk2w": xnk2w.astype(BF),
            "xnv2w": xnv2w.astype(BF), "skipw": skipw,
        })
        in2.append(m)
    res2 = run_bass_kernel_spmd(attn, in2, list(range(NCORE)))

    out = np.zeros((1, C, H, W), np.float32)
    for c in range(NCORE):
        ow = res2.results[c]["outw"]
        for j, (Xi, Yi) in enumerate(wins[c]):
            out[0, :, 16 * Xi:16 * Xi + 16, 16 * Yi:16 * Yi + 16] = \
                ow[:, j].reshape(128, 16, 16)
    return out


# revision 18
# speedup vs baseline: 1.0172x; 1.0172x over previous
"""CrossViewSwapAttention Trainium2 kernel.

Strategy (8 NeuronCores, SPMD, no collectives):
  Launch 1 (prep, sharded by feature rows / BEV rows):
    - geometry embeddings (img_embed, bev_embed) via folded 4xC matmuls
    - BN+ReLU+1x1conv for key/value features (BN folded into conv weights)
    - width-axis LayerNorm (stats per (ch,row) via bn_stats; its affine
      vanishes inside the following feature LayerNorm)
    - feature-dim LayerNorm "core" (x-mean)/std via ones-matmul broadcast
      trick (mean/E[x^2] broadcast over partitions by a 128x128 ones lhsT)
    -> outputs normalized K / V / Q token tensors (bf16, channel-major)
  Host: reshard tokens into per-core attention windows (4 windows/core).
  Launch 2 (windowed attention, sharded by (x,y) windows):
    - per window: project q/k/v (LN gains + tau folded into weights),
      scores^T = kh^T qh via head-row-tiled matmuls (4 heads share the PE),
      exp on ScalarE straight out of PSUM (one op per [128, 4*512] block),
      Z via ones-matmul column sums (col-tiled by head), PV col-tiled by
      head, normalize with reciprocal_approx_fast, out-proj with the
      mean-over-views folded into PSUM accumulation, MLPs, final LN.

All matmuls bf16 (fp32 runs at 1/4 rate on the PE); stats/normalization
paths fp32. All parameter-side algebra (LN gains, BN, tau, biases through
projections) is folded on the host; per-pixel/token compute is on-device.
"""

import numpy as np
from contextlib import ExitStack

import ml_dtypes

import concourse.bass as bass
import concourse.bacc as bacc
import concourse.tile as tile
from concourse import mybir
from concourse.bass_utils import run_bass_kernel_spmd

F32 = mybir.dt.float32
BF16 = mybir.dt.bfloat16
AF = mybir.ActivationFunctionType
ALU = mybir.AluOpType
BF = ml_dtypes.bfloat16

# problem constants
NV = 6
C = 128
FH, FW = 32, 128
IMG_H, IMG_W = 512, 1408
H, W = 64, 128
HEADS, DH = 4, 32
NCORE = 8
EPS = 1e-5

# per-core shard sizes
RPC = FH // NCORE            # 4 feature rows per view per core
HPC = H // NCORE             # 8 bev rows per core
WPC = 4                      # windows per core
NQ1 = NV * 256               # 1536 q tokens per layer-1 window
NK = NV * 128                # 768 k tokens per window
NQ2 = 256                    # deduped q tokens per layer-2 window


# ---------------------------------------------------------------------------
# device-side helpers
# ---------------------------------------------------------------------------

def _featln(nc, pools, x_sb, n_tok, out_sb, ones_neg, ones_pos, chunk=512,
            mult_gp=False):
    """out = (x - mean(x)) / sqrt(var(x) + EPS), mean/var over the 128
    partitions (feature dim), per token (free dim). x_sb/out_sb: [128, n_tok].
    ones_neg/ones_pos: [128,128] bf16 lhsT tiles of -1/128, +1/128. Stats are
    computed on a bf16 copy (1-cycle-per-row matmuls); the normalization
    itself reads the original x. mult_gp routes the final multiply to the
    (otherwise idle) GpSimd engine."""
    sb, lnp = pools["sb_ln"], pools["ps_ln"]
    CH = chunk
    for i in range(0, n_tok, CH):
        n = min(CH, n_tok - i)
        xc = x_sb[:, i:i + n]
        if x_sb.dtype == BF16:
            xb = xc
        else:
            xbt = sb.tile([128, CH], BF16, tag="ln_xb")
            nc.vector.tensor_copy(out=xbt[:, :n], in_=xc)
            xb = xbt[:, :n]
        sq = sb.tile([128, CH], BF16, tag="ln_sq")
        nc.scalar.square(sq[:, :n], xb)
        negm = lnp.tile([128, CH], F32, tag="sm" if CH <= 512 else "ln_negm")
        esq = lnp.tile([128, CH], F32, tag="sm" if CH <= 512 else "ln_esq")
        for j in range(0, n, 512):
            m = min(512, n - j)
            nc.tensor.matmul(negm[:, j:j + m], ones_neg, xb[:, j:j + m],
                             start=True, stop=True)
            nc.tensor.matmul(esq[:, j:j + m], ones_pos, sq[:, j:j + m],
                             start=True, stop=True)
        msq = sb.tile([128, CH], F32, tag="ln_msq")
        nc.scalar.square(msq[:, :n], negm[:, :n])
        # nve = m^2 - eps - E[x^2] = -(var + eps)
        nve = sb.tile([128, CH], F32, tag="ln_nve")
        nc.vector.scalar_tensor_tensor(
            out=nve[:, :n], in0=msq[:, :n], scalar=EPS, in1=esq[:, :n],
            op0=ALU.subtract, op1=ALU.subtract)
        rv = sb.tile([128, CH], F32, tag="ln_rv")
        nc.vector.reciprocal_approx_fast(out=rv[:, :n], in_=nve[:, :n])
        rsig = sb.tile([128, CH], F32, tag="ln_rsig")
        nc.scalar.activation(out=rsig[:, :n], in_=rv[:, :n], func=AF.Sqrt,
                             scale=-1.0)
        cent = sb.tile([128, CH], F32, tag="ln_cent")
        nc.vector.tensor_tensor(out=cent[:, :n], in0=xc, in1=negm[:, :n],
                                op=ALU.add)
        eng = nc.gpsimd if mult_gp else nc.vector
        eng.tensor_tensor(out=out_sb[:, i:i + n], in0=cent[:, :n],
                          in1=rsig[:, :n], op=ALU.mult)


def _mlp(nc, pools, x_sb, n_tok, wa, ba, wb, bb, ones_neg_f, ones_pos_f,
         out_sb):
    """out = x + W2^T gelu(W1^T LN(x) + b1) + b2 (residual included).
    x_sb f32 [128, n_tok]; wa [128,2,128] bf16; ba [128,2]; wb [128,2,128];
    bb [128,1]."""
    sb, sm = pools["sb_ln"], pools["ps_sm"]
    xn = sb.tile([128, n_tok], BF16, tag="mlp_xn")
    _featln(nc, pools, x_sb, n_tok, xn, ones_neg_f, ones_pos_f)
    h_sb = sb.tile([128, 2, n_tok], BF16, tag="mlp_h")
    for j in range(2):
        for sp in range(0, n_tok, 512):
            m = min(512, n_tok - sp)
            hp = sm.tile([128, 512], F32, tag="sm")
            nc.tensor.matmul(hp[:, :m], wa[:, j, :], xn[:, sp:sp + m],
                             start=True, stop=True)
            nc.scalar.activation(out=h_sb[:, j, sp:sp + m], in_=hp[:, :m],
                                 func=AF.Gelu, bias=ba[:, j:j + 1], scale=1.0)
    for sp in range(0, n_tok, 512):
        m = min(512, n_tok - sp)
        yp = sm.tile([128, 512], F32, tag="sm")
        for j in range(2):
            nc.tensor.matmul(yp[:, :m], wb[:, j, :], h_sb[:, j, sp:sp + m],
                             start=(j == 0), stop=(j == 1))
        nc.vector.scalar_tensor_tensor(
            out=out_sb[:, sp:sp + m], in0=yp[:, :m], scalar=bb,
            in1=x_sb[:, sp:sp + m], op0=ALU.add, op1=ALU.add)


# ---------------------------------------------------------------------------
# launch 1: prep
# ---------------------------------------------------------------------------

def build_prep():
    nc = bacc.Bacc("TRN2", target_bir_lowering=False, debug=False,
                   num_devices=NCORE)
    d = {}
    def di(name, shape, dt):
        d[name] = nc.dram_tensor(name, shape, dt, kind="ExternalInput").ap()
    def do(name, shape, dt):
        d[name] = nc.dram_tensor(name, shape, dt, kind="ExternalOutput").ap()

    di("feat", [128, NV * RPC, FW], F32)      # feature rows (v-major)
    di("geomw", [4, NV, 128], F32)            # per-view [A_v; b_v - c_v] lhsT
    di("pixaug", [4, RPC, FW], F32)           # [x*1408, y*512, 1, 1] rows
    di("wbevt", [3, NV, 128], BF16)           # per view [W_bev^T; b_bev-c_v]
    di("gridaug", [3, HPC * W], BF16)         # [grid0 rows; ones]
    di("xrows", [128, HPC, W], F32)           # x, this core's bev rows
    di("betafp", [128, 1], F32)
    di("betafl", [128, 1], F32)
    di("wfp", [128, 128], BF16)               # (W_fp * a_bn).T lhsT
    di("wfl", [128, 128], BF16)
    di("gatet", [128, 1], F32)                # embed_gate per partition

    do("xnq", [128, NV, HPC, W], BF16)
    do("xnk", [128, NV * RPC, FW], BF16)
    do("xnv", [128, NV * RPC, FW], BF16)

    with tile.TileContext(nc) as tc:
        with ExitStack() as ctx:
            _prep_body(ctx, tc, d)
    nc.compile()
    return nc


def _prep_body(ctx, tc, d):
    nc = tc.nc
    const = ctx.enter_context(tc.tile_pool(name="const", bufs=1))
    sb = ctx.enter_context(tc.tile_pool(name="sb", bufs=1))
    sb2 = ctx.enter_context(tc.tile_pool(name="sb2", bufs=2))
    pp = ctx.enter_context(tc.tile_pool(name="pp", bufs=2, space="PSUM"))
    lnp = ctx.enter_context(tc.tile_pool(name="lnp", bufs=1, space="PSUM"))
    pools = {"sb_ln": sb2, "ps_ln": lnp, "ps_sm": pp}

    # constants
    ones1_b = const.tile([128, 128], BF16)
    nc.vector.memset(ones1_b, 1.0)
    oneg_b = const.tile([128, 128], BF16)
    nc.vector.memset(oneg_b, -1.0 / 128)
    opos_b = const.tile([128, 128], BF16)
    nc.vector.memset(opos_b, 1.0 / 128)

    cnst = {}
    for nm in ("geomw", "pixaug", "wbevt", "gridaug", "xrows",
               "betafp", "betafl", "wfp", "wfl", "gatet", "feat"):
        ap = d[nm]
        cnst[nm] = const.tile(list(ap.shape), ap.dtype, name=nm + "_sb")
        nc.sync.dma_start(out=cnst[nm], in_=ap)

    # ---- BEV / query side -------------------------------------------------
    # per view: bev_v = W_bev@grid + (b_bev - c_v), normalize, + x
    NPOS = HPC * W  # 1024 positions
    q = sb.tile([128, NV, NPOS], BF16)
    xflat = cnst["xrows"].rearrange("p a b -> p (a b)")
    for v in range(NV):
        wps = pp.tile([128, NPOS], F32, tag="pp")
        for j in range(0, NPOS, 512):
            nc.tensor.matmul(wps[:, j:j + 512], cnst["wbevt"][:, v, :],
                             cnst["gridaug"][:, j:j + 512],
                             start=True, stop=True)
        sqv = sb2.tile([128, NPOS], BF16, tag="bev_sq")
        nc.scalar.square(sqv, wps)
        n2 = pp.tile([128, NPOS], F32, tag="pp")
        for j in range(0, NPOS, 512):
            nc.tensor.matmul(n2[:, j:j + 512], ones1_b, sqv[:, j:j + 512],
                             start=True, stop=True)
        rn2 = sb2.tile([128, NPOS], F32, tag="bev_rn2")
        nc.vector.reciprocal_approx_fast(out=rn2, in_=n2)
        rn = sb2.tile([128, NPOS], F32, tag="bev_rn")
        nc.scalar.activation(out=rn, in_=rn2, func=AF.Sqrt, scale=1.0)
        qp = sb2.tile([128, NPOS], BF16, tag="bev_qp")
        nc.vector.tensor_tensor(out=qp, in0=wps, in1=rn, op=ALU.mult)
        nc.gpsimd.tensor_tensor(out=q[:, v, :], in0=qp, in1=xflat,
                                op=ALU.add)
    xnq = sb.tile([128, NV * NPOS], BF16)
    _featln(nc, pools, q.rearrange("p a b -> p (a b)"), NV * NPOS, xnq,
            oneg_b, opos_b, chunk=1024, mult_gp=True)
    nc.sync.dma_start(out=d["xnq"],
                      in_=xnq.rearrange("p (a b c) -> p a b c", b=HPC, c=W))

    # ---- image / key / value side ----------------------------------------
    # d_embed per view rows, normalized -> img (bf16)
    img = sb.tile([128, NV, RPC, FW], BF16)
    for v in range(NV):
        dps = pp.tile([128, RPC, FW], F32, tag="pp")
        for r in range(RPC):
            nc.tensor.matmul(dps[:, r, :], cnst["geomw"][:, v, :],
                             cnst["pixaug"][:, r, :], start=True, stop=True)
        dpsf = dps.rearrange("p a b -> p (a b)")
        sqi = sb2.tile([128, RPC * FW], BF16, tag="img_sq")
        nc.scalar.square(sqi, dpsf)
        n2i = pp.tile([128, RPC * FW], F32, tag="pp")
        nc.tensor.matmul(n2i, ones1_b, sqi, start=True, stop=True)
        rn2i = sb2.tile([128, RPC * FW], F32, tag="img_rn2")
        nc.vector.reciprocal_approx_fast(out=rn2i, in_=n2i)
        rni = sb2.tile([128, RPC * FW], F32, tag="img_rn")
        nc.scalar.activation(out=rni, in_=rn2i, func=AF.Sqrt, scale=1.0)
        nc.vector.tensor_tensor(
            out=img[:, v, :, :].rearrange("p a b -> p (a b)"),
            in0=dpsf, in1=rni, op=ALU.mult)

    # relu(feat + beta) -> bf16
    NPX = NV * RPC * FW  # 3072
    featf = cnst["feat"].rearrange("p a b -> p (a b)")
    tfp = sb.tile([128, NPX], BF16)
    nc.vector.tensor_scalar(out=tfp, in0=featf, scalar1=cnst["betafp"],
                            scalar2=0.0, op0=ALU.add, op1=ALU.max)
    tfl = sb.tile([128, NPX], BF16)
    nc.vector.tensor_scalar(out=tfl, in0=featf, scalar1=cnst["betafl"],
                            scalar2=0.0, op0=ALU.add, op1=ALU.max)

    # convs + img gate -> key_flat / val_flat (bf16)
    imgf = img.rearrange("p a b c -> p (a b c)")
    kf = sb.tile([128, NV * RPC, FW], BF16)
    vf = sb.tile([128, NV * RPC, FW], BF16)
    kff = kf.rearrange("p a b -> p (a b)")
    vff = vf.rearrange("p a b -> p (a b)")
    for j in range(0, NPX, 512):
        kc = pp.tile([128, 512], F32, tag="pp")
        nc.tensor.matmul(kc, cnst["wfp"], tfp[:, j:j + 512], start=True,
                         stop=True)
        nc.vector.scalar_tensor_tensor(
            out=kff[:, j:j + 512], in0=imgf[:, j:j + 512],
            scalar=cnst["gatet"], in1=kc, op0=ALU.mult, op1=ALU.add)
        vc = pp.tile([128, 512], F32, tag="pp")
        nc.tensor.matmul(vc, cnst["wfl"], tfl[:, j:j + 512], start=True,
                         stop=True)
        nc.scalar.copy(out=vff[:, j:j + 512], in_=vc)

    # width-axis LN (affine part vanishes into the following feature LN)
    NR = NV * RPC  # 24 rows
    def width_ln(src, dst):
        mv = sb2.tile([128, NR, 2], F32, tag="wln_mv")
        for r in range(NR):
            stats = sb2.tile([128, 6], F32, tag="wln_stats")
            nc.vector.bn_stats(out=stats, in_=src[:, r, :])
            nc.vector.bn_aggr(out=mv[:, r, :], in_=stats)
        ve = sb2.tile([128, NR], F32, tag="wln_ve")
        nc.vector.tensor_scalar(out=ve, in0=mv[:, :, 1], scalar1=EPS,
                                scalar2=None, op0=ALU.add)
        rv = sb2.tile([128, NR], F32, tag="wln_rv")
        nc.vector.reciprocal_approx_fast(out=rv, in_=ve)
        rs = sb2.tile([128, NR], F32, tag="wln_rs")
        nc.scalar.activation(out=rs, in_=rv, func=AF.Sqrt, scale=1.0)
        for r in range(NR):
            nc.vector.tensor_scalar(
                out=dst[:, r, :], in0=src[:, r, :],
                scalar1=mv[:, r, 0:1], scalar2=rs[:, r:r + 1],
                op0=ALU.subtract, op1=ALU.mult)

    kbar = sb.tile([128, NR, FW], BF16)
    width_ln(kf, kbar)
    vbar = sb.tile([128, NR, FW], BF16)
    width_ln(vf, vbar)

    # feature-dim LN -> outputs
    xnk = sb.tile([128, NV * RPC * FW], BF16)
    _featln(nc, pools, kbar.rearrange("p a b -> p (a b)"), NPX, xnk,
            oneg_b, opos_b, chunk=1024, mult_gp=True)
    nc.sync.dma_start(out=d["xnk"],
                      in_=xnk.rearrange("p (a b) -> p a b", b=FW))
    xnv = sb.tile([128, NV * RPC * FW], BF16)
    _featln(nc, pools, vbar.rearrange("p a b -> p (a b)"), NPX, xnv,
            oneg_b, opos_b, chunk=1024, mult_gp=True)
    nc.sync.dma_start(out=d["xnv"],
                      in_=xnv.rearrange("p (a b) -> p a b", b=FW))


# ---------------------------------------------------------------------------
# launch 2: windowed attention
# ---------------------------------------------------------------------------

def build_attn():
    nc = bacc.Bacc("TRN2", target_bir_lowering=False, debug=False,
                   num_devices=NCORE)
    d = {}
    def di(name, shape, dt):
        d[name] = nc.dram_tensor(name, shape, dt, kind="ExternalInput").ap()
    def do(name, shape, dt):
        d[name] = nc.dram_tensor(name, shape, dt, kind="ExternalOutput").ap()

    di("xnq1w", [128, WPC, NQ1], BF16)
    di("xnk1w", [128, WPC, NK], BF16)
    di("xnv1w", [128, WPC, NK], BF16)
    di("xnk2w", [128, WPC, NK], BF16)
    di("xnv2w", [128, WPC, NK], BF16)
    di("skipw", [128, WPC, 256], F32)
    for nm in ("wq1", "wk1", "wv1", "wp1", "wq2", "wk2", "wv2", "wp2"):
        di(nm, [128, 128], BF16)
    for nm in ("bq1", "bk1", "bp1", "bq2", "bk2", "bp2", "bm1b", "bm2b",
               "postg", "postb"):
        di(nm, [128, 1], F32)
    di("wm1a", [128, 2, 128], BF16)
    di("wm1b", [128, 2, 128], BF16)
    di("wm2a", [128, 2, 128], BF16)
    di("wm2b", [128, 2, 128], BF16)
    di("bm1a", [128, 2], F32)
    di("bm2a", [128, 2], F32)

    do("outw", [128, WPC, 256], F32)

    with tile.TileContext(nc) as tc:
        with ExitStack() as ctx:
            _attn_body(ctx, tc, d)
    nc.compile()
    return nc


def _attn_body(ctx, tc, d):
    nc = tc.nc
    const = ctx.enter_context(tc.tile_pool(name="const", bufs=1))
    sb = ctx.enter_context(tc.tile_pool(name="sb", bufs=1))
    sb2 = ctx.enter_context(tc.tile_pool(name="sb2", bufs=2))
    win = ctx.enter_context(tc.tile_pool(name="win", bufs=2))
    ptp = ctx.enter_context(tc.tile_pool(name="ptp", bufs=2))
    qk = ctx.enter_context(tc.tile_pool(name="qk", bufs=1, space="PSUM"))
    # one PSUM pool for everything non-QK: featLN stats tiles share the same
    # [128,512] slots as z/pv/projection psums (bufs=4 = 4 banks; qk = 4)
    sm = ctx.enter_context(tc.tile_pool(name="sm", bufs=4, space="PSUM"))
    pools = {"sb_ln": sb2, "ps_ln": sm, "ps_sm": sm}

    cw = {}
    for nm, ap in d.items():
        if nm in ("outw",):
            continue
        cw[nm] = const.tile(list(ap.shape), ap.dtype, name=nm + "_sb")
        nc.sync.dma_start(out=cw[nm], in_=ap)
    ones1_b = const.tile([128, 128], BF16)
    nc.vector.memset(ones1_b, 1.0)
    oneg_f = const.tile([128, 128], BF16)
    nc.vector.memset(oneg_f, -1.0 / 128)
    opos_f = const.tile([128, 128], BF16)
    nc.vector.memset(opos_f, 1.0 / 128)

    q2all = sb.tile([128, WPC * 256], F32)

    # ---------------- layer 1 windows ----------------
    for w in range(WPC):
        an = _attention(nc, pools, sm, qk, ptp, win, ones1_b,
                        cw["xnq1w"][:, w, :], cw["xnk1w"][:, w, :],
                        cw["xnv1w"][:, w, :], NQ1,
                        cw["wq1"], cw["bq1"], cw["wk1"], cw["bk1"], cw["wv1"])
        # out-proj with mean over views folded into PSUM accumulation
        zm = sm.tile([128, 512], F32, tag="sm")
        for v in range(NV):
            nc.tensor.matmul(zm[:, :256], cw["wp1"],
                             an[:, v * 256:(v + 1) * 256],
                             start=(v == 0), stop=(v == NV - 1))
        nc.vector.scalar_tensor_tensor(
            out=q2all[:, w * 256:(w + 1) * 256], in0=zm[:, :256],
            scalar=cw["bp1"], in1=cw["skipw"][:, w, :],
            op0=ALU.add, op1=ALU.add)

    # ---------------- MLP 1 ----------------
    q2p = sb.tile([128, WPC * 256], F32)
    _mlp(nc, pools, q2all, WPC * 256, cw["wm1a"], cw["bm1a"], cw["wm1b"],
         cw["bm1b"], oneg_f, opos_f, q2p)

    # ---------------- layer 2 windows ----------------
    xnq2 = sb.tile([128, WPC * 256], BF16)
    _featln(nc, pools, q2p, WPC * 256, xnq2, oneg_f, opos_f)

    q3all = sb.tile([128, WPC * 256], F32)
    for w in range(WPC):
        an = _attention(nc, pools, sm, qk, ptp, win, ones1_b,
                        xnq2[:, w * 256:(w + 1) * 256],
                        cw["xnk2w"][:, w, :], cw["xnv2w"][:, w, :], NQ2,
                        cw["wq2"], cw["bq2"], cw["wk2"], cw["bk2"], cw["wv2"])
        zm = sm.tile([128, 512], F32, tag="sm")
        nc.tensor.matmul(zm[:, :256], cw["wp2"], an, start=True, stop=True)
        nc.vector.scalar_tensor_tensor(
            out=q3all[:, w * 256:(w + 1) * 256], in0=zm[:, :256],
            scalar=cw["bp2"], in1=q2p[:, w * 256:(w + 1) * 256],
            op0=ALU.add, op1=ALU.add)

    # ---------------- MLP 2 + post LN ----------------
    q3p = sb.tile([128, WPC * 256], F32)
    _mlp(nc, pools, q3all, WPC * 256, cw["wm2a"], cw["bm2a"], cw["wm2b"],
         cw["bm2b"], oneg_f, opos_f, q3p)

    xn3 = sb.tile([128, WPC * 256], F32)
    _featln(nc, pools, q3p, WPC * 256, xn3, oneg_f, opos_f)
    outw = sb.tile([128, WPC * 256], F32)
    nc.vector.tensor_scalar(out=outw, in0=xn3, scalar1=cw["postg"],
                            scalar2=cw["postb"], op0=ALU.mult, op1=ALU.add)
    nc.sync.dma_start(out=d["outw"],
                      in_=outw.rearrange("p (a b) -> p a b", b=256))


def _attention(nc, pools, sm, qk, ptp, win, ones1_b,
               xnq_sb, xnk_sb, xnv_sb, nq, wq, bq, wk, bk, wv):
    """One window of cross attention. Returns an [128, nq] bf16 tile with the
    normalized per-head attention output (channel-major, heads stacked)."""
    # qh = wq^T xnq + bq  (bf16, [128 hd, nq])
    qh = win.tile([128, nq], BF16, tag="qh")
    for sp in range(0, nq, 512):
        m = min(512, nq - sp)
        qp = sm.tile([128, 512], F32, tag="sm")
        nc.tensor.matmul(qp[:, :m], wq, xnq_sb[:, sp:sp + m],
                         start=True, stop=True)
        nc.vector.tensor_scalar(out=qh[:, sp:sp + m], in0=qp[:, :m],
                                scalar1=bq, scalar2=None, op0=ALU.add)
    # kh = wk^T xnk + bk
    kh = win.tile([128, NK], BF16, tag="kh")
    for sp in range(0, NK, 384):
        kp = sm.tile([128, 512], F32, tag="sm")
        nc.tensor.matmul(kp[:, :384], wk, xnk_sb[:, sp:sp + 384],
                         start=True, stop=True)
        nc.vector.tensor_scalar(out=kh[:, sp:sp + 384], in0=kp[:, :384],
                                scalar1=bk, scalar2=None, op0=ALU.add)
    # vh token-major: [128 tok, kc, hd]
    nkc = NK // 128  # 6 key chunks
    vh = win.tile([128, nkc, 128], BF16, tag="vh")
    for kc in range(nkc):
        vp = sm.tile([128, 512], F32, tag="sm")
        nc.tensor.matmul(vp[:, :128], xnv_sb[:, kc * 128:(kc + 1) * 128], wv,
                         start=True, stop=True)
        nc.vector.tensor_copy(out=vh[:, kc, :], in_=vp[:, :128])

    span = 512 if nq >= 512 else nq
    nsp = (nq + span - 1) // span
    an = win.tile([128, nq], BF16, tag="an")
    for s in range(nsp):
        q0 = s * span
        # scores^T -> exp(P^T) bf16, [128 k, kc, head, span]
        pt = ptp.tile([128, nkc, HEADS, span], BF16, tag="pt")
        for kc in range(nkc):
            for hp in range(2):
                sc = qk.tile([128, 2, 512], F32, tag="qk", bufs=2)
                for hh in range(2):
                    h = 2 * hp + hh
                    hs = slice(32 * h, 32 * h + 32)
                    nc.tensor.matmul(
                        sc[:, hh, :span],
                        kh[hs, kc * 128:(kc + 1) * 128],
                        qh[hs, q0:q0 + span],
                        start=True, stop=True, tile_position=(32 * h, 0))
                nc.scalar.activation(out=pt[:, kc, 2 * hp:2 * hp + 2, :],
                                     in_=sc[:, :, :span], func=AF.Exp)
        # Z (col-tiled ones-matmul) and PV (col-tiled by head)
        zp = sm.tile([128, 512], F32, tag="sm")
        pv = sm.tile([128, 512], F32, tag="sm")
        for kc in range(nkc):
            for h in range(HEADS):
                op = slice(32 * h, 32 * h + 32)
                nc.tensor.matmul(zp[op, :span], ones1_b[:, 0:32],
                                 pt[:, kc, h, :], start=(kc == 0),
                                 stop=(kc == nkc - 1),
                                 tile_position=(0, 32 * h))
                nc.tensor.matmul(pv[op, :span], vh[:, kc, op],
                                 pt[:, kc, h, :], start=(kc == 0),
                                 stop=(kc == nkc - 1),
                                 tile_position=(0, 32 * h))
        rz = win.tile([128, 512], F32, tag="rz")
        nc.vector.reciprocal_approx_fast(out=rz[:, :span], in_=zp[:, :span])
        nc.vector.tensor_tensor(out=an[:, q0:q0 + span], in0=pv[:, :span],
                                in1=rz[:, :span], op=ALU.mult)
    return an


# ---------------------------------------------------------------------------
# host orchestration
# ---------------------------------------------------------------------------

_PROGS = {}


def _progs():
    if "prep" not in _PROGS:
        _PROGS["prep"] = build_prep()
        _PROGS["attn"] = build_attn()
    return _PROGS["prep"], _PROGS["attn"]


def kernel(index, x, grid0, feature, I_inv, E_inv, object_count, params):
    p = {}
    for k, v in params.items():
        if k in ("a1", "a2", "mlp1", "mlp2"):
            p[k] = {kk: np.asarray(vv, dtype=np.float32)
                    for kk, vv in v.items()}
        else:
            p[k] = np.asarray(v, dtype=np.float32)
    x = np.asarray(x, dtype=np.float32)
    grid0 = np.asarray(grid0, dtype=np.float32)
    feature = np.asarray(feature, dtype=np.float32)
    I_inv = np.asarray(I_inv, dtype=np.float32)
    E_inv = np.asarray(E_inv, dtype=np.float32)

    oc = float(np.asarray(object_count).reshape(-1)[0])
    tau = float(np.clip(2.0 / (5.0 + max(oc, 0.0)) + 0.6, 0.4, 1.5))
    s = (DH ** -0.5) / tau

    prep, attn = _progs()

    # ---- geometry folds ----
    Wi, Wc = p["W_img"], p["W_cam"]
    geomw = np.zeros((4, NV, 128), np.float32)
    wbevt = np.zeros((3, NV, 128), np.float32)
    for v in range(NV):
        Ai = Wi @ E_inv[0, v, :, :3] @ I_inv[0, v]         # (128,3)
        bi = Wi @ E_inv[0, v, :, 3]                        # (128,)
        cv = Wc @ E_inv[0, v, :, 3]
        geomw[:3, v, :] = Ai.T
        geomw[3, v, :] = bi - cv
        wbevt[:2, v, :] = p["W_bev"].T
        wbevt[2, v, :] = p["b_bev"] - cv
    wbevt = wbevt.astype(BF)

    xs = np.linspace(0.0, 1.0, FW, dtype=np.float32) * IMG_W
    ys = np.linspace(0.0, 1.0, FH, dtype=np.float32) * IMG_H

    afp = p["bn_fp_g"] / np.sqrt(p["bn_fp_v"] + EPS)
    bfp = (p["bn_fp_b"] - p["bn_fp_m"] * afp) / afp
    afl = p["bn_fl_g"] / np.sqrt(p["bn_fl_v"] + EPS)
    bfl = (p["bn_fl_b"] - p["bn_fl_m"] * afl) / afl
    wfp = np.ascontiguousarray((p["W_fp"] * afp[None, :]).T).astype(BF)
    wfl = np.ascontiguousarray((p["W_fl"] * afl[None, :]).T).astype(BF)
    gate = float(p["embed_gate"])

    # ---- launch 1 ----
    in1 = []
    for c in range(NCORE):
        fh0 = RPC * c
        h0 = HPC * c
        pixaug = np.zeros((4, RPC, FW), np.float32)
        pixaug[0] = xs[None, :]
        pixaug[1] = ys[fh0:fh0 + RPC, None]
        pixaug[2] = 1.0
        pixaug[3] = 1.0
        gridaug = np.concatenate(
            [grid0[:, h0:h0 + HPC, :].reshape(2, -1),
             np.ones((1, HPC * W), np.float32)], 0).astype(BF)
        m = {
            "feat": np.ascontiguousarray(
                feature[0, :, :, fh0:fh0 + RPC, :].transpose(1, 0, 2, 3)
            ).reshape(128, NV * RPC, FW),
            "geomw": geomw, "pixaug": pixaug,
            "wbevt": wbevt, "gridaug": gridaug,
            "xrows": np.ascontiguousarray(x[0, :, h0:h0 + HPC, :]),
            "betafp": bfp.reshape(128, 1), "betafl": bfl.reshape(128, 1),
            "wfp": wfp, "wfl": wfl,
            "gatet": np.full((128, 1), gate, np.float32),
        }
        in1.append(m)
    res1 = run_bass_kernel_spmd(prep, in1, list(range(NCORE)))

    xnk = np.zeros((128, NV, FH, FW), np.float32)
    xnv = np.zeros((128, NV, FH, FW), np.float32)
    xnq = np.zeros((128, NV, H, W), np.float32)
    for c in range(NCORE):
        r = res1.results[c]
        xnk[:, :, RPC * c:RPC * (c + 1), :] = \
            r["xnk"].reshape(128, NV, RPC, FW).astype(np.float32)
        xnv[:, :, RPC * c:RPC * (c + 1), :] = \
            r["xnv"].reshape(128, NV, RPC, FW).astype(np.float32)
        xnq[:, :, HPC * c:HPC * (c + 1), :] = r["xnq"].astype(np.float32)

    # ---- attention weight folds ----
    def attw(a):
        wq = ((a["qln_g"][:, None] * a["Wq"]) * s).astype(BF)
        bqv = ((a["qln_b"] @ a["Wq"] + a["bq"]) * s).astype(np.float32)
        wk = (a["kln_g"][:, None] * a["Wk"]).astype(BF)
        bkv = (a["kln_b"] @ a["Wk"] + a["bk"]).astype(np.float32)
        wvm = (a["vln_g"][:, None] * a["Wv"]).astype(BF)
        bvv = (a["vln_b"] @ a["Wv"] + a["bv"]).astype(np.float32)
        bpv = (bvv @ a["Wp"] + a["bp"]).astype(np.float32)
        return wq, bqv, wk, bkv, wvm, bpv

    wq1, bq1, wk1, bk1, wv1, bp1 = attw(p["a1"])
    wq2, bq2, wk2, bk2, wv2, bp2 = attw(p["a2"])

    def mlpw(mp, g, b):
        wa = (g[:, None] * mp["W1"]).reshape(128, 2, 128).astype(BF)
        ba = np.ascontiguousarray(
            (b @ mp["W1"] + mp["b1"]).reshape(2, 128).T).astype(np.float32)
        wb = np.ascontiguousarray(
            mp["W2"].reshape(2, 128, 128).transpose(1, 0, 2)).astype(BF)
        bb = mp["b2"].reshape(128, 1).astype(np.float32)
        return wa, ba, wb, bb

    wm1a, bm1a, wm1b, bm1b = mlpw(p["mlp1"], p["pre1_g"], p["pre1_b"])
    wm2a, bm2a, wm2b, bm2b = mlpw(p["mlp2"], p["pre2_g"], p["pre2_b"])

    shared2 = {
        "wq1": wq1, "wk1": wk1, "wv1": wv1,
        "wp1": (p["a1"]["Wp"] / NV).astype(BF),
        "wq2": wq2, "wk2": wk2, "wv2": wv2,
        "wp2": p["a2"]["Wp"].astype(BF),
        "bq1": bq1.reshape(128, 1), "bk1": bk1.reshape(128, 1),
        "bp1": bp1.reshape(128, 1),
        "bq2": bq2.reshape(128, 1), "bk2": bk2.reshape(128, 1),
        "bp2": bp2.reshape(128, 1),
        "wm1a": wm1a, "wm1b": wm1b, "wm2a": wm2a, "wm2b": wm2b,
        "bm1a": bm1a, "bm2a": bm2a, "bm1b": bm1b, "bm2b": bm2b,
        "postg": p["post_g"].reshape(128, 1).astype(np.float32),
        "postb": p["post_b"].reshape(128, 1).astype(np.float32),
    }

    in2 = []
    wins = []
    for c in range(NCORE):
        xnq1w = np.zeros((128, WPC, NQ1), np.float32)
        xnk1w = np.zeros((128, WPC, NK), np.float32)
        xnv1w = np.zeros((128, WPC, NK), np.float32)
        xnk2w = np.zeros((128, WPC, NK), np.float32)
        xnv2w = np.zeros((128, WPC, NK), np.float32)
        skipw = np.zeros((128, WPC, 256), np.float32)
        cwins = []
        for j in range(WPC):
            wlin = WPC * c + j
            Xi, Yi = wlin // 8, wlin % 8
            cwins.append((Xi, Yi))
            xnq1w[:, j] = xnq[:, :, 16 * Xi:16 * Xi + 16,
                              16 * Yi:16 * Yi + 16].reshape(128, NQ1)
            xnk1w[:, j] = xnk[:, :, 8 * Xi:8 * Xi + 8,
                              16 * Yi:16 * Yi + 16].reshape(128, NK)
            xnv1w[:, j] = xnv[:, :, 8 * Xi:8 * Xi + 8,
                              16 * Yi:16 * Yi + 16].reshape(128, NK)
            xnk2w[:, j] = xnk[:, :, Xi::4, Yi::8].reshape(128, NK)
            xnv2w[:, j] = xnv[:, :, Xi::4, Yi::8].reshape(128, NK)
            skipw[:, j] = x[0, :, 16 * Xi:16 * Xi + 16,
                            16 * Yi:16 * Yi + 16].reshape(128, 256)
        wins.append(cwins)
        m = dict(shared2)
        m.update({
            "xnq1w": xnq1w.astype(BF), "xnk1w": xnk1w.astype(BF),
            "xnv1w": xnv1w.astype(BF), "xnk2w": xnk2w.astype(BF),
            "xnv2w": xnv2w.astype(BF), "skipw": skipw,
        })
        in2.append(m)
    res2 = run_bass_kernel_spmd(attn, in2, list(range(NCORE)))

    out = np.zeros((1, C, H, W), np.float32)
    for c in range(NCORE):
        ow = res2.results[c]["outw"]
        for j, (Xi, Yi) in enumerate(wins[c]):
            out[0, :, 16 * Xi:16 * Xi + 16, 16 * Yi:16 * Yi + 16] = \
                ow[:, j].reshape(128, 16, 16)
    return out


# revision 20
# speedup vs baseline: 1.0422x; 1.0246x over previous
"""CrossViewSwapAttention Trainium2 kernel.

Strategy (8 NeuronCores, SPMD, no collectives):
  Launch 1 (prep, sharded by feature rows / BEV rows):
    - geometry embeddings (img_embed, bev_embed) via folded 4xC matmuls
    - BN+ReLU+1x1conv for key/value features (BN folded into conv weights)
    - width-axis LayerNorm (stats per (ch,row) via bn_stats; its affine
      vanishes inside the following feature LayerNorm)
    - feature-dim LayerNorm "core" (x-mean)/std via ones-matmul broadcast
      trick (mean/E[x^2] broadcast over partitions by a 128x128 ones lhsT)
    -> outputs normalized K / V / Q token tensors (bf16, channel-major)
  Host: reshard tokens into per-core attention windows (4 windows/core).
  Launch 2 (windowed attention, sharded by (x,y) windows):
    - per window: project q/k/v (LN gains + tau folded into weights),
      scores^T = kh^T qh via head-row-tiled matmuls (4 heads share the PE),
      exp on ScalarE straight out of PSUM (one op per [128, 4*512] block),
      Z via ones-matmul column sums (col-tiled by head), PV col-tiled by
      head, normalize with reciprocal_approx_fast, out-proj with the
      mean-over-views folded into PSUM accumulation, MLPs, final LN.

All matmuls bf16 (fp32 runs at 1/4 rate on the PE); stats/normalization
paths fp32. All parameter-side algebra (LN gains, BN, tau, biases through
projections) is folded on the host; per-pixel/token compute is on-device.
"""

import numpy as np
from contextlib import ExitStack

import ml_dtypes

import concourse.bass as bass
import concourse.bacc as bacc
import concourse.tile as tile
from concourse import mybir
from concourse.bass_utils import run_bass_kernel_spmd

F32 = mybir.dt.float32
BF16 = mybir.dt.bfloat16
AF = mybir.ActivationFunctionType
ALU = mybir.AluOpType
BF = ml_dtypes.bfloat16

# problem constants
NV = 6
C = 128
FH, FW = 32, 128
IMG_H, IMG_W = 512, 1408
H, W = 64, 128
HEADS, DH = 4, 32
NCORE = 8
EPS = 1e-5

# per-core shard sizes
RPC = FH // NCORE            # 4 feature rows per view per core
HPC = H // NCORE             # 8 bev rows per core
WPC = 4                      # windows per core
NQ1 = NV * 256               # 1536 q tokens per layer-1 window
NK = NV * 128                # 768 k tokens per window
NQ2 = 256                    # deduped q tokens per layer-2 window


# ---------------------------------------------------------------------------
# device-side helpers
# ---------------------------------------------------------------------------

def _featln(nc, pools, x_sb, n_tok, out_sb, ones_neg, ones_pos, chunk=512,
            mult_gp=False):
    """out = (x - mean(x)) / sqrt(var(x) + EPS), mean/var over the 128
    partitions (feature dim), per token (free dim). x_sb/out_sb: [128, n_tok].
    ones_neg/ones_pos: [128,128] bf16 lhsT tiles of -1/128, +1/128. Stats are
    computed on a bf16 copy (1-cycle-per-row matmuls); the normalization
    itself reads the original x. mult_gp routes the final multiply to the
    (otherwise idle) GpSimd engine."""
    sb, lnp = pools["sb_ln"], pools["ps_ln"]
    CH = chunk
    for i in range(0, n_tok, CH):
        n = min(CH, n_tok - i)
        xc = x_sb[:, i:i + n]
        if x_sb.dtype == BF16:
            xb = xc
        else:
            xbt = sb.tile([128, CH], BF16, tag="ln_xb")
            nc.vector.tensor_copy(out=xbt[:, :n], in_=xc)
            xb = xbt[:, :n]
        sq = sb.tile([128, CH], BF16, tag="ln_sq")
        nc.scalar.square(sq[:, :n], xb)
        negm = lnp.tile([128, CH], F32, tag="sm" if CH <= 512 else "pp")
        esq = lnp.tile([128, CH], F32, tag="sm" if CH <= 512 else "pp")
        for j in range(0, n, 512):
            m = min(512, n - j)
            nc.tensor.matmul(negm[:, j:j + m], ones_neg, xb[:, j:j + m],
                             start=True, stop=True)
            nc.tensor.matmul(esq[:, j:j + m], ones_pos, sq[:, j:j + m],
                             start=True, stop=True)
        msq = sb.tile([128, CH], F32, tag="ln_msq")
        nc.scalar.square(msq[:, :n], negm[:, :n])
        # nve = m^2 - eps - E[x^2] = -(var + eps)
        nve = sb.tile([128, CH], F32, tag="ln_nve")
        nc.vector.scalar_tensor_tensor(
            out=nve[:, :n], in0=msq[:, :n], scalar=EPS, in1=esq[:, :n],
            op0=ALU.subtract, op1=ALU.subtract)
        rv = sb.tile([128, CH], F32, tag="ln_rv")
        nc.vector.reciprocal_approx_fast(out=rv[:, :n], in_=nve[:, :n])
        rsig = sb.tile([128, CH], F32, tag="ln_rsig")
        nc.scalar.activation(out=rsig[:, :n], in_=rv[:, :n], func=AF.Sqrt,
                             scale=-1.0)
        cent = sb.tile([128, CH], F32, tag="ln_cent")
        nc.vector.tensor_tensor(out=cent[:, :n], in0=xc, in1=negm[:, :n],
                                op=ALU.add)
        eng = nc.gpsimd if mult_gp else nc.vector
        eng.tensor_tensor(out=out_sb[:, i:i + n], in0=cent[:, :n],
                          in1=rsig[:, :n], op=ALU.mult)


def _mlp(nc, pools, x_sb, n_tok, wa, ba, wb, bb, ones_neg_f, ones_pos_f,
         out_sb):
    """out = x + W2^T gelu(W1^T LN(x) + b1) + b2 (residual included).
    x_sb f32 [128, n_tok]; wa [128,2,128] bf16; ba [128,2]; wb [128,2,128];
    bb [128,1]."""
    sb, sm = pools["sb_ln"], pools["ps_sm"]
    xn = sb.tile([128, n_tok], BF16, tag="mlp_xn")
    _featln(nc, pools, x_sb, n_tok, xn, ones_neg_f, ones_pos_f)
    h_sb = sb.tile([128, 2, n_tok], BF16, tag="mlp_h")
    for j in range(2):
        for sp in range(0, n_tok, 512):
            m = min(512, n_tok - sp)
            hp = sm.tile([128, 512], F32, tag="sm")
            nc.tensor.matmul(hp[:, :m], wa[:, j, :], xn[:, sp:sp + m],
                             start=True, stop=True)
            nc.scalar.activation(out=h_sb[:, j, sp:sp + m], in_=hp[:, :m],
                                 func=AF.Gelu, bias=ba[:, j:j + 1], scale=1.0)
    for sp in range(0, n_tok, 512):
        m = min(512, n_tok - sp)
        yp = sm.tile([128, 512], F32, tag="sm")
        for j in range(2):
            nc.tensor.matmul(yp[:, :m], wb[:, j, :], h_sb[:, j, sp:sp + m],
                             start=(j == 0), stop=(j == 1))
        nc.vector.scalar_tensor_tensor(
            out=out_sb[:, sp:sp + m], in0=yp[:, :m], scalar=bb,
            in1=x_sb[:, sp:sp + m], op0=ALU.add, op1=ALU.add)


# ---------------------------------------------------------------------------
# launch 1: prep
# ---------------------------------------------------------------------------

def build_prep():
    nc = bacc.Bacc("TRN2", target_bir_lowering=False, debug=False,
                   num_devices=NCORE)
    d = {}
    def di(name, shape, dt):
        d[name] = nc.dram_tensor(name, shape, dt, kind="ExternalInput").ap()
    def do(name, shape, dt):
        d[name] = nc.dram_tensor(name, shape, dt, kind="ExternalOutput").ap()

    di("feat", [128, NV * RPC, FW], F32)      # feature rows (v-major)
    di("geomw", [4, NV, 128], F32)            # per-view [A_v; b_v - c_v] lhsT
    di("pixaug", [4, RPC, FW], F32)           # [x*1408, y*512, 1, 1] rows
    di("wbevt", [3, NV, 128], BF16)           # per view [W_bev^T; b_bev-c_v]
    di("gridaug", [3, HPC * W], BF16)         # [grid0 rows; ones]
    di("xrows", [128, HPC, W], F32)           # x, this core's bev rows
    di("betafp", [128, 1], F32)
    di("betafl", [128, 1], F32)
    di("wfp", [128, 128], BF16)               # (W_fp * a_bn).T lhsT
    di("wfl", [128, 128], BF16)
    di("gatet", [128, 1], F32)                # embed_gate per partition

    do("xnq", [128, NV, HPC, W], BF16)
    do("xnk", [128, NV * RPC, FW], BF16)
    do("xnv", [128, NV * RPC, FW], BF16)

    with tile.TileContext(nc) as tc:
        with ExitStack() as ctx:
            _prep_body(ctx, tc, d)
    nc.compile()
    return nc


def _prep_body(ctx, tc, d):
    nc = tc.nc
    const = ctx.enter_context(tc.tile_pool(name="const", bufs=1))
    sb = ctx.enter_context(tc.tile_pool(name="sb", bufs=1))
    sb2 = ctx.enter_context(tc.tile_pool(name="sb2", bufs=2))
    # single shared PSUM pool: 4 slots x [128,1024] = 8 banks; featLN stats,
    # geometry, conv and bev psums all rotate through the same slots
    pp = ctx.enter_context(tc.tile_pool(name="pp", bufs=4, space="PSUM"))
    pools = {"sb_ln": sb2, "ps_ln": pp, "ps_sm": pp}

    # constants
    ones1_b = const.tile([128, 128], BF16)
    nc.vector.memset(ones1_b, 1.0)
    oneg_b = const.tile([128, 128], BF16)
    nc.vector.memset(oneg_b, -1.0 / 128)
    opos_b = const.tile([128, 128], BF16)
    nc.vector.memset(opos_b, 1.0 / 128)

    cnst = {}
    for nm in ("geomw", "pixaug", "wbevt", "gridaug", "xrows",
               "betafp", "betafl", "wfp", "wfl", "gatet", "feat"):
        ap = d[nm]
        cnst[nm] = const.tile(list(ap.shape), ap.dtype, name=nm + "_sb")
        nc.sync.dma_start(out=cnst[nm], in_=ap)

    # ---- BEV / query side -------------------------------------------------
    # per view: bev_v = W_bev@grid + (b_bev - c_v), normalize, + x
    NPOS = HPC * W  # 1024 positions
    q = sb.tile([128, NV, NPOS], BF16)
    xflat = cnst["xrows"].rearrange("p a b -> p (a b)")
    for v in range(NV):
        wps = pp.tile([128, NPOS], F32, tag="pp")
        for j in range(0, NPOS, 512):
            nc.tensor.matmul(wps[:, j:j + 512], cnst["wbevt"][:, v, :],
                             cnst["gridaug"][:, j:j + 512],
                             start=True, stop=True)
        sqv = sb2.tile([128, NPOS], BF16, tag="bev_sq")
        nc.scalar.square(sqv, wps)
        n2 = pp.tile([128, NPOS], F32, tag="pp")
        for j in range(0, NPOS, 512):
            nc.tensor.matmul(n2[:, j:j + 512], ones1_b, sqv[:, j:j + 512],
                             start=True, stop=True)
        rn2 = sb2.tile([128, NPOS], F32, tag="bev_rn2")
        nc.vector.reciprocal_approx_fast(out=rn2, in_=n2)
        rn = sb2.tile([128, NPOS], F32, tag="bev_rn")
        nc.scalar.activation(out=rn, in_=rn2, func=AF.Sqrt, scale=1.0)
        qp = sb2.tile([128, NPOS], BF16, tag="bev_qp")
        nc.vector.tensor_tensor(out=qp, in0=wps, in1=rn, op=ALU.mult)
        nc.gpsimd.tensor_tensor(out=q[:, v, :], in0=qp, in1=xflat,
                                op=ALU.add)
    xnq = sb.tile([128, NV * NPOS], BF16)
    _featln(nc, pools, q.rearrange("p a b -> p (a b)"), NV * NPOS, xnq,
            oneg_b, opos_b, chunk=1024, mult_gp=True)
    nc.sync.dma_start(out=d["xnq"],
                      in_=xnq.rearrange("p (a b c) -> p a b c", b=HPC, c=W))

    # ---- image / key / value side ----------------------------------------
    # d_embed per view rows, normalized -> img (bf16)
    img = sb.tile([128, NV, RPC, FW], BF16)
    for v in range(NV):
        dps = pp.tile([128, RPC, FW], F32, tag="pp")
        for r in range(RPC):
            nc.tensor.matmul(dps[:, r, :], cnst["geomw"][:, v, :],
                             cnst["pixaug"][:, r, :], start=True, stop=True)
        dpsf = dps.rearrange("p a b -> p (a b)")
        sqi = sb2.tile([128, RPC * FW], BF16, tag="img_sq")
        nc.scalar.square(sqi, dpsf)
        n2i = pp.tile([128, RPC * FW], F32, tag="pp")
        nc.tensor.matmul(n2i, ones1_b, sqi, start=True, stop=True)
        rn2i = sb2.tile([128, RPC * FW], F32, tag="img_rn2")
        nc.vector.reciprocal_approx_fast(out=rn2i, in_=n2i)
        rni = sb2.tile([128, RPC * FW], F32, tag="img_rn")
        nc.scalar.activation(out=rni, in_=rn2i, func=AF.Sqrt, scale=1.0)
        nc.vector.tensor_tensor(
            out=img[:, v, :, :].rearrange("p a b -> p (a b)"),
            in0=dpsf, in1=rni, op=ALU.mult)

    # relu(feat + beta) -> bf16
    NPX = NV * RPC * FW  # 3072
    featf = cnst["feat"].rearrange("p a b -> p (a b)")
    tfp = sb.tile([128, NPX], BF16)
    nc.vector.tensor_scalar(out=tfp, in0=featf, scalar1=cnst["betafp"],
                            scalar2=0.0, op0=ALU.add, op1=ALU.max)
    tfl = sb.tile([128, NPX], BF16)
    nc.vector.tensor_scalar(out=tfl, in0=featf, scalar1=cnst["betafl"],
                            scalar2=0.0, op0=ALU.add, op1=ALU.max)

    # convs + img gate -> key_flat / val_flat (bf16)
    imgf = img.rearrange("p a b c -> p (a b c)")
    kf = sb.tile([128, NV * RPC, FW], BF16)
    vf = sb.tile([128, NV * RPC, FW], BF16)
    kff = kf.rearrange("p a b -> p (a b)")
    vff = vf.rearrange("p a b -> p (a b)")
    for j in range(0, NPX, 512):
        kc = pp.tile([128, 512], F32, tag="pp")
        nc.tensor.matmul(kc, cnst["wfp"], tfp[:, j:j + 512], start=True,
                         stop=True)
        nc.vector.scalar_tensor_tensor(
            out=kff[:, j:j + 512], in0=imgf[:, j:j + 512],
            scalar=cnst["gatet"], in1=kc, op0=ALU.mult, op1=ALU.add)
        vc = pp.tile([128, 512], F32, tag="pp")
        nc.tensor.matmul(vc, cnst["wfl"], tfl[:, j:j + 512], start=True,
                         stop=True)
        nc.scalar.copy(out=vff[:, j:j + 512], in_=vc)

    # width-axis LN (affine part vanishes into the following feature LN)
    NR = NV * RPC  # 24 rows
    def width_ln(src, dst):
        mv = sb2.tile([128, NR, 2], F32, tag="wln_mv")
        for r in range(NR):
            stats = sb2.tile([128, 6], F32, tag="wln_stats")
            nc.vector.bn_stats(out=stats, in_=src[:, r, :])
            nc.vector.bn_aggr(out=mv[:, r, :], in_=stats)
        ve = sb2.tile([128, NR], F32, tag="wln_ve")
        nc.vector.tensor_scalar(out=ve, in0=mv[:, :, 1], scalar1=EPS,
                                scalar2=None, op0=ALU.add)
        rv = sb2.tile([128, NR], F32, tag="wln_rv")
        nc.vector.reciprocal_approx_fast(out=rv, in_=ve)
        rs = sb2.tile([128, NR], F32, tag="wln_rs")
        nc.scalar.activation(out=rs, in_=rv, func=AF.Sqrt, scale=1.0)
        for r in range(NR):
            nc.vector.tensor_scalar(
                out=dst[:, r, :], in0=src[:, r, :],
                scalar1=mv[:, r, 0:1], scalar2=rs[:, r:r + 1],
                op0=ALU.subtract, op1=ALU.mult)

    kbar = sb.tile([128, NR, FW], BF16)
    width_ln(kf, kbar)
    vbar = sb.tile([128, NR, FW], BF16)
    width_ln(vf, vbar)

    # feature-dim LN -> outputs
    xnk = sb.tile([128, NV * RPC * FW], BF16)
    _featln(nc, pools, kbar.rearrange("p a b -> p (a b)"), NPX, xnk,
            oneg_b, opos_b, chunk=1024, mult_gp=True)
    nc.sync.dma_start(out=d["xnk"],
                      in_=xnk.rearrange("p (a b) -> p a b", b=FW))
    xnv = sb.tile([128, NV * RPC * FW], BF16)
    _featln(nc, pools, vbar.rearrange("p a b -> p (a b)"), NPX, xnv,
            oneg_b, opos_b, chunk=1024, mult_gp=True)
    nc.sync.dma_start(out=d["xnv"],
                      in_=xnv.rearrange("p (a b) -> p a b", b=FW))


# ---------------------------------------------------------------------------
# launch 2: windowed attention
# ---------------------------------------------------------------------------

def build_attn():
    nc = bacc.Bacc("TRN2", target_bir_lowering=False, debug=False,
                   num_devices=NCORE)
    d = {}
    def di(name, shape, dt):
        d[name] = nc.dram_tensor(name, shape, dt, kind="ExternalInput").ap()
    def do(name, shape, dt):
        d[name] = nc.dram_tensor(name, shape, dt, kind="ExternalOutput").ap()

    di("xnq1w", [128, WPC, NQ1], BF16)
    di("xnk1w", [128, WPC, NK], BF16)
    di("xnv1w", [128, WPC, NK], BF16)
    di("xnk2w", [128, WPC, NK], BF16)
    di("xnv2w", [128, WPC, NK], BF16)
    di("skipw", [128, WPC, 256], F32)
    for nm in ("wq1", "wk1", "wv1", "wp1", "wq2", "wk2", "wv2", "wp2"):
        di(nm, [128, 128], BF16)
    for nm in ("bq1", "bk1", "bp1", "bq2", "bk2", "bp2", "bm1b", "bm2b",
               "postg", "postb"):
        di(nm, [128, 1], F32)
    di("wm1a", [128, 2, 128], BF16)
    di("wm1b", [128, 2, 128], BF16)
    di("wm2a", [128, 2, 128], BF16)
    di("wm2b", [128, 2, 128], BF16)
    di("bm1a", [128, 2], F32)
    di("bm2a", [128, 2], F32)

    do("outw", [128, WPC, 256], F32)

    with tile.TileContext(nc) as tc:
        with ExitStack() as ctx:
            _attn_body(ctx, tc, d)
    nc.compile()
    return nc


def _attn_body(ctx, tc, d):
    nc = tc.nc
    const = ctx.enter_context(tc.tile_pool(name="const", bufs=1))
    sb = ctx.enter_context(tc.tile_pool(name="sb", bufs=1))
    sb2 = ctx.enter_context(tc.tile_pool(name="sb2", bufs=2))
    win = ctx.enter_context(tc.tile_pool(name="win", bufs=2))
    ptp = ctx.enter_context(tc.tile_pool(name="ptp", bufs=2))
    qk = ctx.enter_context(tc.tile_pool(name="qk", bufs=1, space="PSUM"))
    # one PSUM pool for everything non-QK: featLN stats tiles share the same
    # [128,512] slots as z/pv/projection psums (bufs=4 = 4 banks; qk = 4)
    sm = ctx.enter_context(tc.tile_pool(name="sm", bufs=4, space="PSUM"))
    pools = {"sb_ln": sb2, "ps_ln": sm, "ps_sm": sm}

    cw = {}
    for nm, ap in d.items():
        if nm in ("outw",):
            continue
        cw[nm] = const.tile(list(ap.shape), ap.dtype, name=nm + "_sb")
        nc.sync.dma_start(out=cw[nm], in_=ap)
    ones1_b = const.tile([128, 128], BF16)
    nc.vector.memset(ones1_b, 1.0)
    oneg_f = const.tile([128, 128], BF16)
    nc.vector.memset(oneg_f, -1.0 / 128)
    opos_f = const.tile([128, 128], BF16)
    nc.vector.memset(opos_f, 1.0 / 128)

    q2all = sb.tile([128, WPC * 256], F32)

    # ---------------- layer 1 windows ----------------
    for w in range(WPC):
        an = _attention(nc, pools, sm, qk, ptp, win, ones1_b,
                        cw["xnq1w"][:, w, :], cw["xnk1w"][:, w, :],
                        cw["xnv1w"][:, w, :], NQ1,
                        cw["wq1"], cw["bq1"], cw["wk1"], cw["bk1"], cw["wv1"])
        # out-proj with mean over views folded into PSUM accumulation
        zm = sm.tile([128, 512], F32, tag="sm")
        for v in range(NV):
            nc.tensor.matmul(zm[:, :256], cw["wp1"],
                             an[:, v * 256:(v + 1) * 256],
                             start=(v == 0), stop=(v == NV - 1))
        nc.vector.scalar_tensor_tensor(
            out=q2all[:, w * 256:(w + 1) * 256], in0=zm[:, :256],
            scalar=cw["bp1"], in1=cw["skipw"][:, w, :],
            op0=ALU.add, op1=ALU.add)

    # ---------------- MLP 1 ----------------
    q2p = sb.tile([128, WPC * 256], F32)
    _mlp(nc, pools, q2all, WPC * 256, cw["wm1a"], cw["bm1a"], cw["wm1b"],
         cw["bm1b"], oneg_f, opos_f, q2p)

    # ---------------- layer 2 windows ----------------
    xnq2 = sb.tile([128, WPC * 256], BF16)
    _featln(nc, pools, q2p, WPC * 256, xnq2, oneg_f, opos_f)

    q3all = sb.tile([128, WPC * 256], F32)
    for w in range(WPC):
        an = _attention(nc, pools, sm, qk, ptp, win, ones1_b,
                        xnq2[:, w * 256:(w + 1) * 256],
                        cw["xnk2w"][:, w, :], cw["xnv2w"][:, w, :], NQ2,
                        cw["wq2"], cw["bq2"], cw["wk2"], cw["bk2"], cw["wv2"])
        zm = sm.tile([128, 512], F32, tag="sm")
        nc.tensor.matmul(zm[:, :256], cw["wp2"], an, start=True, stop=True)
        nc.vector.scalar_tensor_tensor(
            out=q3all[:, w * 256:(w + 1) * 256], in0=zm[:, :256],
            scalar=cw["bp2"], in1=q2p[:, w * 256:(w + 1) * 256],
            op0=ALU.add, op1=ALU.add)

    # ---------------- MLP 2 + post LN ----------------
    q3p = sb.tile([128, WPC * 256], F32)
    _mlp(nc, pools, q3all, WPC * 256, cw["wm2a"], cw["bm2a"], cw["wm2b"],
         cw["bm2b"], oneg_f, opos_f, q3p)

    xn3 = sb.tile([128, WPC * 256], F32)
    _featln(nc, pools, q3p, WPC * 256, xn3, oneg_f, opos_f)
    outw = sb.tile([128, WPC * 256], F32)
    nc.vector.tensor_scalar(out=outw, in0=xn3, scalar1=cw["postg"],
                            scalar2=cw["postb"], op0=ALU.mult, op1=ALU.add)
    nc.sync.dma_start(out=d["outw"],
                      in_=outw.rearrange("p (a b) -> p a b", b=256))


def _attention(nc, pools, sm, qk, ptp, win, ones1_b,
               xnq_sb, xnk_sb, xnv_sb, nq, wq, bq, wk, bk, wv):
    """One window of cross attention. Returns an [128, nq] bf16 tile with the
    normalized per-head attention output (channel-major, heads stacked)."""
    # qh = wq^T xnq + bq  (bf16, [128 hd, nq])
    qh = win.tile([128, nq], BF16, tag="qh")
    for sp in range(0, nq, 512):
        m = min(512, nq - sp)
        qp = sm.tile([128, 512], F32, tag="sm")
        nc.tensor.matmul(qp[:, :m], wq, xnq_sb[:, sp:sp + m],
                         start=True, stop=True)
        nc.vector.tensor_scalar(out=qh[:, sp:sp + m], in0=qp[:, :m],
                                scalar1=bq, scalar2=None, op0=ALU.add)
    # kh = wk^T xnk + bk
    kh = win.tile([128, NK], BF16, tag="kh")
    for sp in range(0, NK, 384):
        kp = sm.tile([128, 512], F32, tag="sm")
        nc.tensor.matmul(kp[:, :384], wk, xnk_sb[:, sp:sp + 384],
                         start=True, stop=True)
        nc.vector.tensor_scalar(out=kh[:, sp:sp + 384], in0=kp[:, :384],
                                scalar1=bk, scalar2=None, op0=ALU.add)
    # vh token-major: [128 tok, kc, hd]
    nkc = NK // 128  # 6 key chunks
    vh = win.tile([128, nkc, 128], BF16, tag="vh")
    for kc in range(nkc):
        vp = sm.tile([128, 512], F32, tag="sm")
        nc.tensor.matmul(vp[:, :128], xnv_sb[:, kc * 128:(kc + 1) * 128], wv,
                         start=True, stop=True)
        nc.vector.tensor_copy(out=vh[:, kc, :], in_=vp[:, :128])

    span = 512 if nq >= 512 else nq
    nsp = (nq + span - 1) // span
    an = win.tile([128, nq], BF16, tag="an")
    for s in range(nsp):
        q0 = s * span
        # scores^T -> exp(P^T) bf16, [128 k, kc, head, span]
        pt = ptp.tile([128, nkc, HEADS, span], BF16, tag="pt")
        for kc in range(nkc):
            for hp in range(2):
                sc = qk.tile([128, 2, 512], F32, tag="qk", bufs=2)
                for hh in range(2):
                    h = 2 * hp + hh
                    hs = slice(32 * h, 32 * h + 32)
                    nc.tensor.matmul(
                        sc[:, hh, :span],
                        kh[hs, kc * 128:(kc + 1) * 128],
                        qh[hs, q0:q0 + span],
                        start=True, stop=True, tile_position=(32 * h, 0))
                nc.scalar.activation(out=pt[:, kc, 2 * hp:2 * hp + 2, :],
                                     in_=sc[:, :, :span], func=AF.Exp)
        # Z (col-tiled ones-matmul) and PV (col-tiled by head)
        zp = sm.tile([128, 512], F32, tag="sm")
        pv = sm.tile([128, 512], F32, tag="sm")
        for kc in range(nkc):
            for h in range(HEADS):
                op = slice(32 * h, 32 * h + 32)
                nc.tensor.matmul(zp[op, :span], ones1_b[:, 0:32],
                                 pt[:, kc, h, :], start=(kc == 0),
                                 stop=(kc == nkc - 1),
                                 tile_position=(0, 32 * h))
                nc.tensor.matmul(pv[op, :span], vh[:, kc, op],
                                 pt[:, kc, h, :], start=(kc == 0),
                                 stop=(kc == nkc - 1),
                                 tile_position=(0, 32 * h))
        rz = win.tile([128, 512], F32, tag="rz")
        nc.vector.reciprocal_approx_fast(out=rz[:, :span], in_=zp[:, :span])
        nc.vector.tensor_tensor(out=an[:, q0:q0 + span], in0=pv[:, :span],
                                in1=rz[:, :span], op=ALU.mult)
    return an


# ---------------------------------------------------------------------------
# host orchestration
# ---------------------------------------------------------------------------

_PROGS = {}


def _progs():
    if "prep" not in _PROGS:
        _PROGS["prep"] = build_prep()
        _PROGS["attn"] = build_attn()
    return _PROGS["prep"], _PROGS["attn"]


def kernel(index, x, grid0, feature, I_inv, E_inv, object_count, params):
    p = {}
    for k, v in params.items():
        if k in ("a1", "a2", "mlp1", "mlp2"):
            p[k] = {kk: np.asarray(vv, dtype=np.float32)
                    for kk, vv in v.items()}
        else:
            p[k] = np.asarray(v, dtype=np.float32)
    x = np.asarray(x, dtype=np.float32)
    grid0 = np.asarray(grid0, dtype=np.float32)
    feature = np.asarray(feature, dtype=np.float32)
    I_inv = np.asarray(I_inv, dtype=np.float32)
    E_inv = np.asarray(E_inv, dtype=np.float32)

    oc = float(np.asarray(object_count).reshape(-1)[0])
    tau = float(np.clip(2.0 / (5.0 + max(oc, 0.0)) + 0.6, 0.4, 1.5))
    s = (DH ** -0.5) / tau

    prep, attn = _progs()

    # ---- geometry folds ----
    Wi, Wc = p["W_img"], p["W_cam"]
    geomw = np.zeros((4, NV, 128), np.float32)
    wbevt = np.zeros((3, NV, 128), np.float32)
    for v in range(NV):
        Ai = Wi @ E_inv[0, v, :, :3] @ I_inv[0, v]         # (128,3)
        bi = Wi @ E_inv[0, v, :, 3]                        # (128,)
        cv = Wc @ E_inv[0, v, :, 3]
        geomw[:3, v, :] = Ai.T
        geomw[3, v, :] = bi - cv
        wbevt[:2, v, :] = p["W_bev"].T
        wbevt[2, v, :] = p["b_bev"] - cv
    wbevt = wbevt.astype(BF)

    xs = np.linspace(0.0, 1.0, FW, dtype=np.float32) * IMG_W
    ys = np.linspace(0.0, 1.0, FH, dtype=np.float32) * IMG_H

    afp = p["bn_fp_g"] / np.sqrt(p["bn_fp_v"] + EPS)
    bfp = (p["bn_fp_b"] - p["bn_fp_m"] * afp) / afp
    afl = p["bn_fl_g"] / np.sqrt(p["bn_fl_v"] + EPS)
    bfl = (p["bn_fl_b"] - p["bn_fl_m"] * afl) / afl
    wfp = np.ascontiguousarray((p["W_fp"] * afp[None, :]).T).astype(BF)
    wfl = np.ascontiguousarray((p["W_fl"] * afl[None, :]).T).astype(BF)
    gate = float(p["embed_gate"])

    # ---- launch 1 ----
    in1 = []
    for c in range(NCORE):
        fh0 = RPC * c
        h0 = HPC * c
        pixaug = np.zeros((4, RPC, FW), np.float32)
        pixaug[0] = xs[None, :]
        pixaug[1] = ys[fh0:fh0 + RPC, None]
        pixaug[2] = 1.0
        pixaug[3] = 1.0
        gridaug = np.concatenate(
            [grid0[:, h0:h0 + HPC, :].reshape(2, -1),
             np.ones((1, HPC * W), np.float32)], 0).astype(BF)
        m = {
            "feat": np.ascontiguousarray(
                feature[0, :, :, fh0:fh0 + RPC, :].transpose(1, 0, 2, 3)
            ).reshape(128, NV * RPC, FW),
            "geomw": geomw, "pixaug": pixaug,
            "wbevt": wbevt, "gridaug": gridaug,
            "xrows": np.ascontiguousarray(x[0, :, h0:h0 + HPC, :]),
            "betafp": bfp.reshape(128, 1), "betafl": bfl.reshape(128, 1),
            "wfp": wfp, "wfl": wfl,
            "gatet": np.full((128, 1), gate, np.float32),
        }
        in1.append(m)
    res1 = run_bass_kernel_spmd(prep, in1, list(range(NCORE)))

    xnk = np.zeros((128, NV, FH, FW), np.float32)
    xnv = np.zeros((128, NV, FH, FW), np.float32)
    xnq = np.zeros((128, NV, H, W), np.float32)
    for c in range(NCORE):
        r = res1.results[c]
        xnk[:, :, RPC * c:RPC * (c + 1), :] = \
            r["xnk"].reshape(128, NV, RPC, FW).astype(np.float32)
        xnv[:, :, RPC * c:RPC * (c + 1), :] = \
            r["xnv"].reshape(128, NV, RPC, FW).astype(np.float32)
        xnq[:, :, HPC * c:HPC * (c + 1), :] = r["xnq"].astype(np.float32)

    # ---- attention weight folds ----
    def attw(a):
        wq = ((a["qln_g"][:, None] * a["Wq"]) * s).astype(BF)
        bqv = ((a["qln_b"] @ a["Wq"] + a["bq"]) * s).astype(np.float32)
        wk = (a["kln_g"][:, None] * a["Wk"]).astype(BF)
        bkv = (a["kln_b"] @ a["Wk"] + a["bk"]).astype(np.float32)
        wvm = (a["vln_g"][:, None] * a["Wv"]).astype(BF)
        bvv = (a["vln_b"] @ a["Wv"] + a["bv"]).astype(np.float32)
        bpv = (bvv @ a["Wp"] + a["bp"]).astype(np.float32)
        return wq, bqv, wk, bkv, wvm, bpv

    wq1, bq1, wk1, bk1, wv1, bp1 = attw(p["a1"])
    wq2, bq2, wk2, bk2, wv2, bp2 = attw(p["a2"])

    def mlpw(mp, g, b):
        wa = (g[:, None] * mp["W1"]).reshape(128, 2, 128).astype(BF)
        ba = np.ascontiguousarray(
            (b @ mp["W1"] + mp["b1"]).reshape(2, 128).T).astype(np.float32)
        wb = np.ascontiguousarray(
            mp["W2"].reshape(2, 128, 128).transpose(1, 0, 2)).astype(BF)
        bb = mp["b2"].reshape(128, 1).astype(np.float32)
        return wa, ba, wb, bb

    wm1a, bm1a, wm1b, bm1b = mlpw(p["mlp1"], p["pre1_g"], p["pre1_b"])
    wm2a, bm2a, wm2b, bm2b = mlpw(p["mlp2"], p["pre2_g"], p["pre2_b"])

    shared2 = {
        "wq1": wq1, "wk1": wk1, "wv1": wv1,
        "wp1": (p["a1"]["Wp"] / NV).astype(BF),
        "wq2": wq2, "wk2": wk2, "wv2": wv2,
        "wp2": p["a2"]["Wp"].astype(BF),
        "bq1": bq1.reshape(128, 1), "bk1": bk1.reshape(128, 1),
        "bp1": bp1.reshape(128, 1),
        "bq2": bq2.reshape(128, 1), "bk2": bk2.reshape(128, 1),
        "bp2": bp2.reshape(128, 1),
        "wm1a": wm1a, "wm1b": wm1b, "wm2a": wm2a, "wm2b": wm2b,
        "bm1a": bm1a, "bm2a": bm2a, "bm1b": bm1b, "bm2b": bm2b,
        "postg": p["post_g"].reshape(128, 1).astype(np.float32),
        "postb": p["post_b"].reshape(128, 1).astype(np.float32),
    }

    in2 = []
    wins = []
    for c in range(NCORE):
        xnq1w = np.zeros((128, WPC, NQ1), np.float32)
        xnk1w = np.zeros((128, WPC, NK), np.float32)
        xnv1w = np.zeros((128, WPC, NK), np.float32)
        xnk2w = np.zeros((128, WPC, NK), np.float32)
        xnv2w = np.zeros((128, WPC, NK), np.float32)
        skipw = np.zeros((128, WPC, 256), np.float32)
        cwins = []
        for j in range(WPC):
            wlin = WPC * c + j
            Xi, Yi = wlin // 8, wlin % 8
            cwins.append((Xi, Yi))
            xnq1w[:, j] = xnq[:, :, 16 * Xi:16 * Xi + 16,
                              16 * Yi:16 * Yi + 16].reshape(128, NQ1)
            xnk1w[:, j] = xnk[:, :, 8 * Xi:8 * Xi + 8,
                              16 * Yi:16 * Yi + 16].reshape(128, NK)
            xnv1w[:, j] = xnv[:, :, 8 * Xi:8 * Xi + 8,
                              16 * Yi:16 * Yi + 16].reshape(128, NK)
            xnk2w[:, j] = xnk[:, :, Xi::4, Yi::8].reshape(128, NK)
            xnv2w[:, j] = xnv[:, :, Xi::4, Yi::8].reshape(128, NK)
            skipw[:, j] = x[0, :, 16 * Xi:16 * Xi + 16,
                            16 * Yi:16 * Yi + 16].reshape(128, 256)
        wins.append(cwins)
        m = dict(shared2)
        m.update({
            "xnq1w": xnq1w.astype(BF), "xnk1w": xnk1w.astype(BF),
            "xnv1w": xnv1w.astype(BF), "xnk2w": xnk2w.astype(BF),
            "xnv2w": xnv2w.astype(BF), "skipw": skipw,
        })
        in2.append(m)
    res2 = run_bass_kernel_spmd(attn, in2, list(range(NCORE)))

    out = np.zeros((1, C, H, W), np.float32)
    for c in range(NCORE):
        ow = res2.results[c]["outw"]
        for j, (Xi, Yi) in enumerate(wins[c]):
            out[0, :, 16 * Xi:16 * Xi + 16, 16 * Yi:16 * Yi + 16] = \
                ow[:, j].reshape(128, 16, 16)
    return out
